# revision 32
# baseline (speedup 1.0000x reference)
"""CAPAttentionModule Trainium2 kernel.

Data-parallel over batch: 8 images -> 8 NeuronCores, one image per core.
Per core (x: [512, 9216] = [C, H*W], H=W=96):
  k1 = relu(Wkp x + b)              [128, HW]   (1x1 conv, BN folded)
  k2 = relu(dw3x3(k1) + b)          [128, HW]   (banded matmuls, see below)
  v1 = relu(Wvp x + b)              [256, HW]
  v2 = relu(dw3x3(v1) + b)          [256, HW]
  key = psp([k1;k2])   [256, 110],  value = psp([v1;v2])  [512, 110]
  q  = relu(Wq x + b)               [256, HW]
  sim = softmax_s(q^T key / 16)     [HW, 110]   (no max-subtract; |sim|<4)
  out = x + value @ sim^T           [512, HW]   (bf16 residual/store)

Depthwise 3x3: instead of 9 diagonal-matmul passes (1/128 PE density),
rows are re-laid out into 12-row stripes across partitions
(p = u*8 + ci holds row h = 12*jb+u-1 of channel ci*16+g, 1-row halos),
so a banded weight matrix computes all 3 dy taps in ONE pass: 3 passes
(dx in {0,1,2}) instead of 9.  The re-layout goes through a DRAM staging
buffer (SBUF->DRAM per 14-row stripe, one DRAM->SBUF read per chunk)
because DMA APs must stay partition-major on both ends.  4x4 pooling of
the dw output: DVE reduces w by 4, then a ones-banded PE matmul sums the
4 row-partitions; tiny DMAs scatter the pooled grid back channel-major.
PSP pooling: 24x24 block-sum grid per map, then small batched reduces
for the 1/3/6/8 grids; normalization (and the 1/sqrt(256) sim scale) is
folded into per-s scale tiles.
"""

import numpy as np

P = 128
HH = 96
WP = 98          # padded width/height (zero border ring)
HW = 9216
HWP = WP * WP    # 9604: [98, 98] with zero border, data at [1:97, 1:97]
RB = 24          # row blocks of 4 rows
RBN = 4 * HH     # 384
NCH = 18         # phase-B column chunks
NCW = 512
S = 110
NB = 112         # banded partitions: u*8+ci, u in [0,14), ci in [0,8)
NJB = 8          # stripes of 12 valid rows
FB = 16 * NJB * WP   # banded free size (g, jb, w) = 12544


def bass_ap_pool_view(ap_rows):
    """[p, >=4*WP] AP at the start of 4 data rows (stride WP) ->
    [p, wq, h, ws] view for a 4x4 pooling reduce over (h, ws)."""
    v = ap_rows[:, 0:4 * WP].rearrange("p (h w) -> p h w", w=WP)
    v = v[:, :, 0:HH]
    return v.rearrange("p h (wq ws) -> p wq h ws", ws=4)


def build_bass():
    import concourse.bacc as bacc
    import concourse.tile as tile
    from concourse import mybir
    from contextlib import ExitStack

    f32 = mybir.dt.float32
    f32r = mybir.dt.float32r
    bf16 = mybir.dt.bfloat16
    AF = mybir.ActivationFunctionType
    AX = mybir.AxisListType

    nc = bacc.Bacc("TRN2", target_bir_lowering=False, debug=False,
                   enable_asserts=False, num_devices=8)

    xb_d = nc.dram_tensor("xb", [512, HW], bf16, kind="ExternalInput").ap()
    wq_d = nc.dram_tensor("wq", [512, 256], bf16, kind="ExternalInput").ap()
    wkp_d = nc.dram_tensor("wkp", [512, 128], bf16, kind="ExternalInput").ap()
    wvp_d = nc.dram_tensor("wvp", [512, 256], bf16, kind="ExternalInput").ap()
    bw_d = nc.dram_tensor("bw", [3, 16, 3, NB, 96], bf16,
                          kind="ExternalInput").ap()
    wp_d = nc.dram_tensor("wp", [96, 24], bf16, kind="ExternalInput").ap()
    bb_d = nc.dram_tensor("bb", [3, 16, 96], f32, kind="ExternalInput").ap()
    id_d = nc.dram_tensor("ident", [128, 128], bf16, kind="ExternalInput").ap()
    scl_d = nc.dram_tensor("scl", [2, 128, S], f32, kind="ExternalInput").ap()
    bias_d = nc.dram_tensor("bias", [128, 8], f32, kind="ExternalInput").ap()
    y_d = nc.dram_tensor("y", [512, HW], bf16, kind="ExternalOutput").ap()
    # dw staging: per chunk [u14][ci8][g16][jb8][w98]
    st_d = nc.dram_tensor("stage", [3, 14 * 8 * 16 * NJB, WP], bf16,
                          kind="Internal").ap()
    st5 = st_d.rearrange("c (u ci g jb) w -> c u ci g jb w",
                         u=14, ci=8, g=16)

    xb_r = xb_d.rearrange("(t p) n -> p t n", p=P)
    y_r = y_d.rearrange("(t p) n -> p t n", p=P)

    with tile.TileContext(nc) as tc:
        with ExitStack() as top:
            cpool = top.enter_context(tc.tile_pool(name="consts", bufs=1))
            kpool = top.enter_context(tc.tile_pool(name="keep", bufs=1))

            # early consts on sync ring (needed by the primary loop)
            c_wkp = cpool.tile([P, 4 * 128], bf16)
            nc.sync.dma_start(c_wkp[:].rearrange("p (t m) -> p t m", t=4),
                              wkp_d.rearrange("(t p) m -> p t m", p=P))
            c_wvp = cpool.tile([P, 4 * 256], bf16)
            nc.sync.dma_start(c_wvp[:].rearrange("p (t m) -> p t m", t=4),
                              wvp_d.rearrange("(t p) m -> p t m", p=P))
            c_bias = cpool.tile([P, 8], f32)
            nc.sync.dma_start(c_bias[:], bias_d)
            # later consts on the scalar ring (don't block x loads)
            c_wq = cpool.tile([P, 4 * 256], bf16)
            nc.scalar.dma_start(c_wq[:].rearrange("p (t m) -> p t m", t=4),
                                wq_d.rearrange("(t p) m -> p t m", p=P))
            c_id = cpool.tile([P, 128], bf16)
            nc.scalar.dma_start(c_id[:], id_d)
            c_scl = cpool.tile([P, 2 * S], f32)
            nc.scalar.dma_start(c_scl[:].rearrange("p (s m) -> p s m", s=2),
                                scl_d.rearrange("s p m -> p s m"))
            c_wp = cpool.tile([96, 24], bf16)
            nc.scalar.dma_start(c_wp[:], wp_d)
            c_bb = cpool.tile([96, 48], f32)
            nc.scalar.dma_start(c_bb[:].rearrange("p (c g) -> p c g", c=3),
                                bb_d.rearrange("c g p -> p c g"))

            keyn = kpool.tile([P, 2 * S], bf16)       # normalized key (incl /16)
            vT = kpool.tile([S, 512], bf16)           # value^T [s, c]

            # ---------------- Phase A: key/value branches ----------------
            with ExitStack() as actx:
                bigp = actx.enter_context(tc.tile_pool(name="bigA", bufs=1))
                xap = actx.enter_context(tc.tile_pool(name="xa", bufs=2))
                tmpp = actx.enter_context(tc.tile_pool(name="tmpA", bufs=1))
                bwp = actx.enter_context(tc.tile_pool(name="bw", bufs=2))
                bdp = actx.enter_context(tc.tile_pool(name="banded", bufs=3))
                rp_ = actx.enter_context(tc.tile_pool(name="rt", bufs=1))
                pgp = actx.enter_context(tc.tile_pool(name="pg", bufs=1))
                blkp = actx.enter_context(tc.tile_pool(name="blkD", bufs=6))

                k1p = bigp.tile([P, HWP], bf16)
                v1p = bigp.tile([P, 2 * HWP], bf16)
                p24 = bigp.tile([P, 6 * 576], f32)
                allp = bigp.tile([P, 6 * S], f32)
                valn = bigp.tile([P, 4 * S], bf16)

                # zero the pad border (rows 0/97, cols 0/97)
                for chv in (k1p[:, 0:HWP], v1p[:, 0:HWP], v1p[:, HWP:2 * HWP]):
                    c3 = chv.rearrange("p (h w) -> p h w", w=WP)
                    nc.gpsimd.memset(c3[:, 0:1, :], 0.0)
                    nc.gpsimd.memset(c3[:, 97:98, :], 0.0)
                    nc.gpsimd.memset(c3[:, 1:97, 0:1], 0.0)
                    nc.gpsimd.memset(c3[:, 1:97, 97:98], 0.0)

                srcs = [k1p, v1p[:, 0:HWP], v1p[:, HWP:2 * HWP]]

                def stage_write(ct, jb):
                    src = srcs[ct].rearrange("c (hp w) -> c hp w", w=WP)
                    src = src[:, 12 * jb:12 * jb + 14, :]
                    dst = st5[ct, :, :, :, jb, :].rearrange(
                        "u ci g w -> (ci g) u w")
                    nc.scalar.dma_start(dst, src)

                # primary 1x1 convs, streamed by 4-row blocks (2 blocks/DMA),
                # with per-block pooling of k1/v1a/v1b interleaved on DVE
                with tc.tile_pool(name="psA", bufs=2, space="PSUM") as psA:
                    for rbb in range(RB // 2):
                        xt = xap.tile([P, 4 * 2 * RBN], bf16, name="xt")
                        nc.sync.dma_start(
                            xt[:].rearrange("p (t n) -> p t n", t=4),
                            xb_r[:, :, rbb * 2 * RBN:(rbb + 1) * 2 * RBN])
                        dsts = [
                            (k1p, 0, c_wkp, 128, 0, 0),
                            (v1p, 0, c_wvp, 256, 2, 2),
                            (v1p, 1, c_wvp, 256, 3, 3),
                        ]
                        for sub in range(2):
                            rb = rbb * 2 + sub
                            for di, (dst, half, wt, wm, bcol, slot) in enumerate(dsts):
                                ps = psA.tile([P, RBN], f32, name=f"pps{di}")
                                for cc in range(4):
                                    lo = cc * wm + (half * 128 if wm == 256 else 0)
                                    nc.tensor.matmul(
                                        ps[:], wt[:, lo:lo + 128],
                                        xt[:, cc * 2 * RBN + sub * RBN:
                                           cc * 2 * RBN + (sub + 1) * RBN],
                                        start=(cc == 0), stop=(cc == 3))
                                dv = dst[:, half * HWP:(half + 1) * HWP].rearrange(
                                    "p (h w) -> p h w", w=WP)
                                nc.scalar.activation(
                                    dv[:, 4 * rb + 1:4 * rb + 5, 1:97],
                                    ps[:].rearrange("p (h w) -> p h w", w=HH),
                                    AF.Relu, bias=c_bias[:, bcol:bcol + 1])
                                st = (4 * rb + 1) * WP + 1
                                pv = dst[:, half * HWP + st:half * HWP + st + 4 * WP]
                                pv = bass_ap_pool_view(pv)
                                nc.vector.reduce_sum(
                                    p24[:, slot * 576 + rb * 24:slot * 576 + (rb + 1) * 24],
                                    pv, axis=AX.XY)
                            # stripe jb completes at rb = 3*jb+3; issue one
                            # stripe late so the sync-ring DMA never waits at
                            # the queue head (its relu has already retired)
                            if rb >= 6 and rb % 3 == 0:
                                jb = (rb - 6) // 3
                                for ct in range(3):
                                    stage_write(ct, jb)
                    for jb in (6, 7):
                        for ct in range(3):
                            stage_write(ct, jb)

                # small pools over a map range [m0, m1) -> allp columns
                def smallpools(m0, m1):
                    m = m1 - m0
                    allp_v = allp[:, m0 * S:m1 * S].rearrange(
                        "p (m s) -> p m s", s=S)
                    p24s = p24[:, m0 * 576:m1 * 576]
                    nc.vector.reduce_sum(
                        allp_v[:, :, 0:1],
                        p24s.rearrange("p (m s) -> p m s", s=576), axis=AX.X)
                    tmp = tmpp.tile([P, 1152], f32, name="tmpx", tag="tmp")
                    nc.vector.reduce_sum(
                        tmp[:, 0:m * 72],
                        p24s.rearrange("p (mh wq ws) -> p mh wq ws", wq=3, ws=8),
                        axis=AX.X)
                    nc.vector.reduce_sum(
                        allp_v[:, :, 1:10],
                        tmp[:, 0:m * 72].rearrange(
                            "p (m hq hs wq) -> p m hq wq hs", m=m, hq=3, hs=8),
                        axis=AX.X)
                    tmp6 = tmpp.tile([P, 1152], f32, name="tmpx", tag="tmp")
                    nc.vector.reduce_sum(
                        tmp6[:, 0:m * 144],
                        p24s.rearrange("p (mh wq ws) -> p mh wq ws", wq=6, ws=4),
                        axis=AX.X)
                    nc.vector.reduce_sum(
                        allp_v[:, :, 10:46],
                        tmp6[:, 0:m * 144].rearrange(
                            "p (m hq hs wq) -> p m hq wq hs", m=m, hq=6, hs=4),
                        axis=AX.X)
                    tmp8 = tmpp.tile([P, 1152], f32, name="tmpx", tag="tmp")
                    nc.vector.reduce_sum(
                        tmp8[:, 0:m * 192],
                        p24s.rearrange("p (mh wq ws) -> p mh wq ws", wq=8, ws=3),
                        axis=AX.X)
                    nc.vector.reduce_sum(
                        allp_v[:, :, 46:110],
                        tmp8[:, 0:m * 192].rearrange(
                            "p (m hq hs wq) -> p m hq wq hs", m=m, hq=8, hs=3),
                        axis=AX.X)

                def vt_build(j):
                    tp = psTp.tile([P, 128], bf16, name="tp", tag="tp")
                    nc.tensor.transpose(tp[0:S, :], valn[:, j * S:(j + 1) * S],
                                        c_id[:])
                    nc.scalar.copy(vT[:, j * 128:(j + 1) * 128], tp[0:S, :])

                def val_finish(m0, m1):
                    smallpools(m0, m1)
                    for mm in range(m0, m1):
                        j = mm - 2
                        nc.vector.tensor_mul(valn[:, j * S:(j + 1) * S],
                                             allp[:, mm * S:(mm + 1) * S],
                                             c_scl[:, S:2 * S])
                        vt_build(j)

                # banded depthwise 3x3 per chunk; pool via DVE w-reduce +
                # ones-banded PE matmul + scatter DMAs back to p24
                with tc.tile_pool(name="psD", bufs=4, space="PSUM") as psD, \
                        tc.tile_pool(name="psP", bufs=2, space="PSUM") as psP, \
                        tc.tile_pool(name="psTa", bufs=2, space="PSUM") as psTp:
                    # maps 2,3 (v1a, v1b) complete after the primary loop
                    val_finish(2, 4)
                    for ct, slot in ((0, 1), (1, 4), (2, 5)):
                        half = FB // 2
                        rd = st5[ct].rearrange("u ci g jb w -> (u ci) (g jb w)")
                        # same ring as stage writes: FIFO gives DRAM w->r order
                        bd0 = bdp.tile([NB, half], bf16, name="bdh")
                        nc.sync.dma_start(bd0[:], rd[:, 0:half])
                        bd1 = bdp.tile([NB, half], bf16, name="bdh")
                        nc.sync.dma_start(bd1[:], rd[:, half:FB])
                        cbw = bwp.tile([NB, 48 * 96], bf16, name="bwt")
                        nc.scalar.dma_start(
                            cbw[:].rearrange("p (g dx m) -> p g dx m",
                                             g=16, dx=3),
                            bw_d[ct].rearrange("g dx p m -> p g dx m"))
                        Rt = rp_.tile([96, 16 * 192], bf16, name="rt")
                        PG = pgp.tile([24, 16 * 192], f32, name="pg")
                        for g in range(16):
                            bv = (bd0 if g < 8 else bd1)[:].rearrange(
                                "p (g jb w) -> p g jb w", jb=NJB, w=WP)
                            for jbh in range(2):
                                ps = psD.tile([96, RBN], f32, name="dwp")
                                for dx in range(3):
                                    nc.tensor.matmul(
                                        ps[:],
                                        cbw[:, (g * 3 + dx) * 96:
                                            (g * 3 + dx + 1) * 96],
                                        bv[:, g % 8, jbh * 4:(jbh + 1) * 4,
                                           dx:dx + HH],
                                        start=(dx == 0), stop=(dx == 2))
                                blk = blkp.tile([96, RBN], bf16, name="blk")
                                nc.scalar.activation(
                                    blk[:], ps[:], AF.Relu,
                                    bias=c_bb[:, ct * 16 + g:ct * 16 + g + 1])
                                with nc.allow_low_precision(
                                        reason="4-col pool sums in bf16"):
                                    nc.vector.reduce_sum(
                                        Rt[:, g * 192 + jbh * 96:
                                           g * 192 + (jbh + 1) * 96],
                                        blk[:].rearrange(
                                            "p (jb wq ws) -> p jb wq ws",
                                            jb=4, ws=4),
                                        axis=AX.X)
                            if g % 2 == 1:
                                pp_ = psP.tile([24, 384], f32, name="poolp")
                                nc.tensor.matmul(
                                    pp_[:], c_wp[:],
                                    Rt[:, (g - 1) * 192:(g + 1) * 192],
                                    start=True, stop=True)
                                nc.scalar.copy(
                                    PG[:, (g - 1) * 192:(g + 1) * 192], pp_[:])
                        # scatter PG -> p24 channel-major (r-major grid)
                        p24s = p24[:, slot * 576:(slot + 1) * 576]
                        for m in range(3):
                            for co in range(8):
                                sa = PG[m * 8 + co:m * 8 + co + 1, :].rearrange(
                                    "o (g jb wq) -> o g jb wq", jb=NJB, wq=24)
                                da = p24s.rearrange(
                                    "c (jb m wq) -> c jb m wq", m=3, wq=24)
                                nc.scalar.dma_start(
                                    da[co * 16:(co + 1) * 16, :, m, :], sa)
                        if ct == 0:
                            # key branch done: pool + normalize immediately so
                            # phase-B sim/softmax can overlap the value chunks
                            smallpools(0, 2)
                            for kq in range(2):
                                nc.vector.tensor_mul(
                                    keyn[:, kq * S:(kq + 1) * S],
                                    allp[:, kq * S:(kq + 1) * S], c_scl[:, 0:S])
                        elif ct == 1:
                            val_finish(4, 5)
                        else:
                            val_finish(5, 6)

            # ---------------- Phase B: query / attention / output ----------------
            with ExitStack() as bctx:
                xqp = bctx.enter_context(tc.tile_pool(name="xq", bufs=5))
                qp = bctx.enter_context(tc.tile_pool(name="qsb", bufs=5))
                pp = bctx.enter_context(tc.tile_pool(name="pexp", bufs=8))
                sp = bctx.enter_context(tc.tile_pool(name="small", bufs=8))
                stp = bctx.enter_context(tc.tile_pool(name="simT", bufs=5))
                obp = bctx.enter_context(tc.tile_pool(name="outb", bufs=3))
                psQ = bctx.enter_context(tc.tile_pool(name="psQ", bufs=1, space="PSUM"))
                psS = bctx.enter_context(tc.tile_pool(name="psS", bufs=2, space="PSUM"))
                psT2 = bctx.enter_context(tc.tile_pool(name="psT2", bufs=2, space="PSUM"))
                psC = bctx.enter_context(tc.tile_pool(name="psC", bufs=2, space="PSUM"))

                for n in range(NCH):
                    xtb = xqp.tile([P, 4 * NCW], bf16, name="xtq")
                    nc.sync.dma_start(
                        xtb[:].rearrange("p (t n) -> p t n", t=4),
                        xb_r[:, :, n * NCW:(n + 1) * NCW])
                    qsb = qp.tile([P, 2 * NCW], bf16, name="qsb")
                    for kq in range(2):
                        qps = psQ.tile([P, NCW], f32, name=f"q{kq}")
                        for cc in range(4):
                            lo = cc * 256 + kq * 128
                            nc.tensor.matmul(
                                qps[:], c_wq[:, lo:lo + 128],
                                xtb[:, cc * NCW:(cc + 1) * NCW],
                                start=(cc == 0), stop=(cc == 3))
                        nc.scalar.activation(qsb[:, kq * NCW:(kq + 1) * NCW],
                                             qps[:], AF.Relu,
                                             bias=c_bias[:, 6 + kq:7 + kq])
                    sT = stp.tile([S, NCW], bf16, name="sT")
                    for ns in range(4):
                        sps = psS.tile([P, S], f32, name="sim")
                        for kq in range(2):
                            nc.tensor.matmul(
                                sps[:],
                                qsb[:, kq * NCW + ns * 128:kq * NCW + (ns + 1) * 128],
                                keyn[:, kq * S:(kq + 1) * S],
                                start=(kq == 0), stop=(kq == 1))
                        pe = pp.tile([P, S], bf16, name="pe")
                        sums = sp.tile([P, 1], f32, name="sums")
                        nc.scalar.activation(pe[:], sps[:], AF.Exp,
                                             accum_out=sums[:])
                        rp = sp.tile([P, 1], f32, name="rp")
                        nc.vector.reciprocal(rp[:], sums[:])
                        pn = pp.tile([P, S], bf16, name="pn")
                        nc.vector.tensor_scalar_mul(pn[:], pe[:], rp[:])
                        tp2 = psT2.tile([P, 128], bf16, name="tp2")
                        nc.tensor.transpose(tp2[0:S, :], pn[:], c_id[:])
                        nc.scalar.copy(sT[:, ns * 128:(ns + 1) * 128], tp2[0:S, :])
                    outb = obp.tile([P, 4 * NCW], bf16, name="outb")
                    for cv in range(4):
                        cps = psC.tile([P, NCW], f32, name="ctx")
                        nc.tensor.matmul(cps[:], vT[:, cv * 128:(cv + 1) * 128],
                                         sT[:], start=True, stop=True)
                        nc.vector.tensor_add(outb[:, cv * NCW:(cv + 1) * NCW],
                                             cps[:],
                                             xtb[:, cv * NCW:(cv + 1) * NCW])
                    # store on the ScalarE HWDGE ring: keeps a resid-delayed
                    # store from head-of-line blocking the sync-ring x loads
                    nc.scalar.dma_start(
                        y_r[:, :, n * NCW:(n + 1) * NCW],
                        outb[:].rearrange("p (t n) -> p t n", t=4))

    nc.compile()
    return nc


def prep_host_inputs(inputs):
    """Fold BN affine into weights, build band/pool/bias aux tensors."""
    g = lambda a: np.ascontiguousarray(np.asarray(a, dtype=np.float32))
    wq = (g(inputs["q_g"])[:, None] * g(inputs["q_w"])[:, :, 0, 0]).T
    wkp = (g(inputs["kp_g"])[:, None] * g(inputs["kp_w"])[:, :, 0, 0]).T
    wvp = (g(inputs["vp_g"])[:, None] * g(inputs["vp_w"])[:, :, 0, 0]).T
    wkc = g(inputs["kc_g"])[:, None, None] * g(inputs["kc_w"])[:, 0]   # [128,3,3]
    wvc = g(inputs["vc_g"])[:, None, None] * g(inputs["vc_w"])[:, 0]   # [256,3,3]

    # banded dw weights: bw[ct, g, kx, u*8+ci, r*8+ci]
    kers = [wkc, wvc[:128], wvc[128:]]
    bw = np.zeros((3, 16, 3, NB, 96), np.float32)
    for ct in range(3):
        ker = kers[ct]
        for gg in range(16):
            for ci in range(8):
                ch = ci * 16 + gg
                for kx in range(3):
                    for r in range(12):
                        for ky in range(3):
                            bw[ct, gg, kx, (r + ky) * 8 + ci, r * 8 + ci] = \
                                ker[ch, ky, kx]

    wp = np.zeros((96, 24), np.float32)
    for r in range(12):
        for co in range(8):
            wp[r * 8 + co, (r // 4) * 8 + co] = 1.0

    dwb = [g(inputs["kc_b"]), g(inputs["vc_b"])[:128], g(inputs["vc_b"])[128:]]
    bb = np.zeros((3, 16, 96), np.float32)
    for ct in range(3):
        for gg in range(16):
            for ci in range(8):
                bb[ct, gg, np.arange(12) * 8 + ci] = dwb[ct][ci * 16 + gg]

    scale110 = np.zeros(S, np.float32)
    scale110[0] = 1.0 / 9216
    scale110[1:10] = 1.0 / 1024
    scale110[10:46] = 1.0 / 256
    scale110[46:110] = 1.0 / 144
    scl = np.zeros((2, 128, S), np.float32)
    scl[0] = scale110 / 16.0
    scl[1] = scale110

    bias = np.zeros((128, 8), np.float32)
    bias[:, 0] = g(inputs["kp_b"])
    bias[:, 2] = g(inputs["vp_b"])[:128]
    bias[:, 3] = g(inputs["vp_b"])[128:]
    bias[:, 6] = g(inputs["q_b"])[:128]
    bias[:, 7] = g(inputs["q_b"])[128:]

    import ml_dtypes
    return {
        "wq": np.ascontiguousarray(wq).astype(ml_dtypes.bfloat16),
        "wkp": np.ascontiguousarray(wkp).astype(ml_dtypes.bfloat16),
        "wvp": np.ascontiguousarray(wvp).astype(ml_dtypes.bfloat16),
        "bw": bw.astype(ml_dtypes.bfloat16),
        "wp": wp.astype(ml_dtypes.bfloat16),
        "bb": bb,
        "ident": np.eye(128, dtype=ml_dtypes.bfloat16),
        "scl": scl,
        "bias": bias,
    }


def make_in_maps(inputs):
    host = prep_host_inputs(inputs)
    x = np.asarray(inputs["x"], dtype=np.float32)
    B = x.shape[0]
    in_maps = []
    import ml_dtypes
    for b in range(B):
        m = dict(host)
        m["xb"] = np.ascontiguousarray(
            x[b].reshape(512, HW)).astype(ml_dtypes.bfloat16)
        in_maps.append(m)
    return in_maps


_NC = None


def get_nc():
    global _NC
    if _NC is None:
        _NC = build_bass()
    return _NC


def kernel(**inputs):
    from concourse import bass_utils
    nc = get_nc()
    in_maps = make_in_maps(inputs)
    res = bass_utils.run_bass_kernel_spmd(
        nc, in_maps, core_ids=list(range(len(in_maps))), trace=False)
    outs = [np.asarray(r["y"], dtype=np.float32).reshape(512, HH, HH)
            for r in res.results]
    return np.stack(outs, axis=0)


# revision 35
# speedup vs baseline: 1.4928x; 1.4928x over previous
"""CAPAttentionModule Trainium2 kernel.

Data-parallel over batch: 8 images -> 8 NeuronCores, one image per core.
Per core (x: [512, 9216] = [C, H*W], H=W=96):
  k1 = relu(Wkp x + b)              [128, HW]   (1x1 conv, BN folded)
  k2 = relu(dw3x3(k1) + b)          [128, HW]   (depthwise via diagonal matmuls)
  v1 = relu(Wvp x + b)              [256, HW]
  v2 = relu(dw3x3(v1) + b)          [256, HW]
  key = psp([k1;k2])   [256, 110],  value = psp([v1;v2])  [512, 110]
  q  = relu(Wq x + b)               [256, HW]
  sim = softmax_s(q^T key / 16)     [HW, 110]   (no max-subtract; |sim|<4)
  out = x + value @ sim^T           [512, HW]

All matmuls use float32r (full-rate fp32 on the PE at N>=256).
Depthwise 3x3 runs as 9 shifted diagonal matmuls accumulating in PSUM;
SAME-padding comes from a zero column pad (width 98 layout) plus
row-restricted APs at the image top/bottom (has_written overwrite
semantics make ragged accumulation exact).
PSP pooling: one 5D strided reduce to a 24x24 sum grid per map, then
small batched reduces for the 1/3/6/8 grids; normalization (and the
1/sqrt(256) sim scale) is folded into per-s scale tiles.
"""

import numpy as np

P = 128
HH = 96
WP = 98          # padded width/height (zero border ring)
HW = 9216
HWP = WP * WP    # 9604: [98, 98] with zero border, data at [1:97, 1:97]
RB = 24          # row blocks of 4 rows
RBN = 4 * HH     # 384
NCH = 18         # phase-B column chunks
NCW = 512
DWG = 6          # dw row-blocks per psum group
S = 110


def _f32r(ap):
    from concourse import mybir
    return ap.bitcast(mybir.dt.float32r)




def bass_ap_pool_view(ap_rows):
    """[p, >=4*WP] AP at the start of 4 data rows (stride WP) ->
    [p, wq, h, ws] view for a 4x4 pooling reduce over (h, ws)."""
    v = ap_rows[:, 0:4 * WP].rearrange("p (h w) -> p h w", w=WP)
    v = v[:, :, 0:HH]
    return v.rearrange("p h (wq ws) -> p wq h ws", ws=4)

def build_bass():
    import concourse.bacc as bacc
    import concourse.tile as tile
    from concourse import mybir
    from contextlib import ExitStack

    f32 = mybir.dt.float32
    f32r = mybir.dt.float32r
    bf16 = mybir.dt.bfloat16
    AF = mybir.ActivationFunctionType
    AX = mybir.AxisListType

    nc = bacc.Bacc("TRN2", target_bir_lowering=False, debug=False,
                   enable_asserts=False, num_devices=8)

    xb_d = nc.dram_tensor("xb", [512, HW], bf16, kind="ExternalInput").ap()
    wq_d = nc.dram_tensor("wq", [512, 256], bf16, kind="ExternalInput").ap()
    wkp_d = nc.dram_tensor("wkp", [512, 128], bf16, kind="ExternalInput").ap()
    wvp_d = nc.dram_tensor("wvp", [512, 256], bf16, kind="ExternalInput").ap()
    diag_d = nc.dram_tensor("diag", [3, 9, 128, 128], bf16, kind="ExternalInput").ap()
    id_d = nc.dram_tensor("ident", [128, 128], bf16, kind="ExternalInput").ap()
    scl_d = nc.dram_tensor("scl", [2, 128, S], f32, kind="ExternalInput").ap()
    bias_d = nc.dram_tensor("bias", [128, 8], f32, kind="ExternalInput").ap()
    y_d = nc.dram_tensor("y", [512, HW], bf16, kind="ExternalOutput").ap()

    xb_r = xb_d.rearrange("(t p) n -> p t n", p=P)
    y_r = y_d.rearrange("(t p) n -> p t n", p=P)

    with tile.TileContext(nc) as tc:
        with ExitStack() as top:
            cpool = top.enter_context(tc.tile_pool(name="consts", bufs=1))
            kpool = top.enter_context(tc.tile_pool(name="keep", bufs=1))

            # consts needed by the primary loop go first on the sync ring so
            # the first x chunks aren't delayed; the rest ride the scalar ring
            c_wkp = cpool.tile([P, 4 * 128], bf16)
            nc.sync.dma_start(c_wkp[:].rearrange("p (t m) -> p t m", t=4),
                              wkp_d.rearrange("(t p) m -> p t m", p=P))
            c_wvp = cpool.tile([P, 4 * 256], bf16)
            nc.sync.dma_start(c_wvp[:].rearrange("p (t m) -> p t m", t=4),
                              wvp_d.rearrange("(t p) m -> p t m", p=P))
            c_bias = cpool.tile([P, 8], f32)
            nc.sync.dma_start(c_bias[:], bias_d)
            c_wq = cpool.tile([P, 4 * 256], bf16)
            nc.scalar.dma_start(c_wq[:].rearrange("p (t m) -> p t m", t=4),
                                wq_d.rearrange("(t p) m -> p t m", p=P))
            c_dg = cpool.tile([P, 27 * 128], bf16)
            nc.scalar.dma_start(c_dg[:].rearrange("p (ct m) -> p ct m", ct=27),
                                diag_d.rearrange("c t p m -> p (c t) m"))
            c_id = cpool.tile([P, 128], bf16)
            nc.scalar.dma_start(c_id[:], id_d)
            c_scl = cpool.tile([P, 2 * S], f32)
            nc.scalar.dma_start(c_scl[:].rearrange("p (s m) -> p s m", s=2),
                                scl_d.rearrange("s p m -> p s m"))

            keyn = kpool.tile([P, 2 * S], bf16)       # normalized key (incl /16)
            vT = kpool.tile([S, 512], bf16)           # value^T [s, c]
            # x (bf16) resident in SBUF for both phases: [p, (t, n)]
            xall = kpool.tile([P, 4 * HW], bf16)
            xall_v = xall[:].rearrange("p (t n) -> p t n", t=4)

            # ---------------- Phase A: key/value branches ----------------
            with ExitStack() as actx:
                bigp = actx.enter_context(tc.tile_pool(name="bigA", bufs=1))
                blkp = actx.enter_context(tc.tile_pool(name="blk", bufs=6))
                tmpp = actx.enter_context(tc.tile_pool(name="tmpA", bufs=1))

                k1p = bigp.tile([P, HWP], bf16)
                v1p = bigp.tile([P, 2 * HWP], bf16)
                p24 = bigp.tile([P, 6 * 576], f32)
                allp = bigp.tile([P, 6 * S], f32)
                valn = bigp.tile([P, 4 * S], bf16)

                # zero the pad border (rows 0/97, cols 0/97)
                for chv in (k1p[:, 0:HWP], v1p[:, 0:HWP], v1p[:, HWP:2 * HWP]):
                    c3 = chv.rearrange("p (h w) -> p h w", w=WP)
                    nc.gpsimd.memset(c3[:, 0:1, :], 0.0)
                    nc.gpsimd.memset(c3[:, 97:98, :], 0.0)
                    nc.gpsimd.memset(c3[:, 1:97, 0:1], 0.0)
                    nc.gpsimd.memset(c3[:, 1:97, 97:98], 0.0)

                # primary 1x1 convs, streamed by 4-row blocks (2 blocks/DMA),
                # with per-block pooling of k1/v1a/v1b interleaved on DVE
                with tc.tile_pool(name="psA", bufs=2, space="PSUM") as psA:
                    for rbb in range(RB // 2):
                        nc.sync.dma_start(
                            xall_v[:, :, rbb * 2 * RBN:(rbb + 1) * 2 * RBN],
                            xb_r[:, :, rbb * 2 * RBN:(rbb + 1) * 2 * RBN])
                        dsts = [
                            (k1p, 0, c_wkp, 128, 0, 0),
                            (v1p, 0, c_wvp, 256, 2, 2),
                            (v1p, 1, c_wvp, 256, 3, 3),
                        ]
                        for sub in range(2):
                            rb = rbb * 2 + sub
                            for di, (dst, half, wt, wm, bcol, slot) in enumerate(dsts):
                                ps = psA.tile([P, RBN], f32, name=f"pps{di}")
                                for cc in range(4):
                                    lo = cc * wm + (half * 128 if wm == 256 else 0)
                                    nc.tensor.matmul(
                                        ps[:], wt[:, lo:lo + 128],
                                        xall[:, cc * HW + rb * RBN:
                                             cc * HW + (rb + 1) * RBN],
                                        start=(cc == 0), stop=(cc == 3))
                                dv = dst[:, half * HWP:(half + 1) * HWP].rearrange(
                                    "p (h w) -> p h w", w=WP)
                                nc.scalar.activation(
                                    dv[:, 4 * rb + 1:4 * rb + 5, 1:97],
                                    ps[:].rearrange("p (h w) -> p h w", w=HH),
                                    AF.Relu, bias=c_bias[:, bcol:bcol + 1])
                                st = (4 * rb + 1) * WP + 1
                                pv = dst[:, half * HWP + st:half * HWP + st + 4 * WP]
                                pv = bass_ap_pool_view(pv)
                                nc.vector.reduce_sum(
                                    p24[:, slot * 576 + rb * 24:slot * 576 + (rb + 1) * 24],
                                    pv, axis=AX.XY)

                # small pools over a map range [m0, m1) -> allp columns
                def smallpools(m0, m1):
                    m = m1 - m0
                    allp_v = allp[:, m0 * S:m1 * S].rearrange(
                        "p (m s) -> p m s", s=S)
                    p24s = p24[:, m0 * 576:m1 * 576]
                    nc.vector.reduce_sum(
                        allp_v[:, :, 0:1],
                        p24s.rearrange("p (m s) -> p m s", s=576), axis=AX.X)
                    tmp = tmpp.tile([P, 1152], f32, name="tmp", tag="tmp")
                    nc.vector.reduce_sum(
                        tmp[:, 0:m * 72],
                        p24s.rearrange("p (mh wq ws) -> p mh wq ws", wq=3, ws=8),
                        axis=AX.X)
                    nc.vector.reduce_sum(
                        allp_v[:, :, 1:10],
                        tmp[:, 0:m * 72].rearrange(
                            "p (m hq hs wq) -> p m hq wq hs", m=m, hq=3, hs=8),
                        axis=AX.X)
                    tmp6 = tmpp.tile([P, 1152], f32, name="tmp6", tag="tmp")
                    nc.vector.reduce_sum(
                        tmp6[:, 0:m * 144],
                        p24s.rearrange("p (mh wq ws) -> p mh wq ws", wq=6, ws=4),
                        axis=AX.X)
                    nc.vector.reduce_sum(
                        allp_v[:, :, 10:46],
                        tmp6[:, 0:m * 144].rearrange(
                            "p (m hq hs wq) -> p m hq wq hs", m=m, hq=6, hs=4),
                        axis=AX.X)
                    tmp8 = tmpp.tile([P, 1152], f32, name="tmp8", tag="tmp")
                    nc.vector.reduce_sum(
                        tmp8[:, 0:m * 192],
                        p24s.rearrange("p (mh wq ws) -> p mh wq ws", wq=8, ws=3),
                        axis=AX.X)
                    nc.vector.reduce_sum(
                        allp_v[:, :, 46:110],
                        tmp8[:, 0:m * 192].rearrange(
                            "p (m hq hs wq) -> p m hq wq hs", m=m, hq=8, hs=3),
                        axis=AX.X)


                # depthwise 3x3 via diagonal matmuls + pooling of k2/v2;
                # value maps pooled/transposed as soon as each is complete
                def vt_build(j):
                    tp = psTp.tile([P, 128], bf16, name="tp", tag="tp")
                    nc.tensor.transpose(tp[0:S, :], valn[:, j * S:(j + 1) * S],
                                        c_id[:])
                    nc.scalar.copy(vT[:, j * 128:(j + 1) * 128], tp[0:S, :])

                def val_finish(m0, m1):
                    smallpools(m0, m1)
                    for mm in range(m0, m1):
                        j = mm - 2
                        nc.vector.tensor_mul(valn[:, j * S:(j + 1) * S],
                                             allp[:, mm * S:(mm + 1) * S],
                                             c_scl[:, S:2 * S])
                        vt_build(j)

                with tc.tile_pool(name="psD", bufs=1, space="PSUM") as psD, \
                        tc.tile_pool(name="psTa", bufs=2, space="PSUM") as psTp:
                    # maps 2,3 (v1a, v1b) complete after the primary loop
                    val_finish(2, 4)
                    chunks = [(k1p[:, 0:HWP], 0, 1, 1),
                              (v1p[:, 0:HWP], 1, 4, 4),
                              (v1p[:, HWP:2 * HWP], 2, 5, 5)]
                    for chv, ci, bcol, slot in chunks:
                        ch3 = chv.rearrange("p (h w) -> p h w", w=WP)
                        for g in range(RB // DWG):
                            pss = [psD.tile([P, RBN], f32, name=f"dw{j}")
                                   for j in range(DWG)]
                            for t in range(9):
                                dy, dx = t // 3, t % 3
                                dgap = c_dg[:, (ci * 9 + t) * 128:(ci * 9 + t + 1) * 128]
                                for j in range(DWG):
                                    r0 = (g * DWG + j) * 4
                                    rhs = ch3[:, r0 + dy:r0 + dy + 4, dx:dx + HH]
                                    nc.tensor.matmul(
                                        pss[j][:], dgap, rhs,
                                        start=(t == 0), stop=(t == 8))
                            for j in range(DWG):
                                rb = g * DWG + j
                                blk = blkp.tile([P, RBN], bf16, name="blk")
                                nc.scalar.activation(
                                    blk[:], pss[j][:], AF.Relu,
                                    bias=c_bias[:, bcol:bcol + 1])
                                bv = blk[:].rearrange(
                                    "p (h wq ws) -> p wq h ws", h=4, ws=4)
                                nc.vector.reduce_sum(
                                    p24[:, slot * 576 + rb * 24:slot * 576 + (rb + 1) * 24],
                                    bv, axis=AX.XY)
                        if ci == 0:
                            # key branch done: pool + normalize immediately so
                            # phase-B sim/softmax can overlap the value chunks
                            smallpools(0, 2)
                            for kq in range(2):
                                nc.vector.tensor_mul(
                                    keyn[:, kq * S:(kq + 1) * S],
                                    allp[:, kq * S:(kq + 1) * S], c_scl[:, 0:S])
                        elif ci == 1:
                            val_finish(4, 5)
                        else:
                            val_finish(5, 6)


            # ---------------- Phase B: query / attention / output ----------------
            with ExitStack() as bctx:
                qp = bctx.enter_context(tc.tile_pool(name="qsb", bufs=5))
                pp = bctx.enter_context(tc.tile_pool(name="pexp", bufs=8))
                sp = bctx.enter_context(tc.tile_pool(name="small", bufs=8))
                stp = bctx.enter_context(tc.tile_pool(name="simT", bufs=5))
                obp = bctx.enter_context(tc.tile_pool(name="outb", bufs=3))
                psQ = bctx.enter_context(tc.tile_pool(name="psQ", bufs=1, space="PSUM"))
                psS = bctx.enter_context(tc.tile_pool(name="psS", bufs=2, space="PSUM"))
                psT2 = bctx.enter_context(tc.tile_pool(name="psT2", bufs=2, space="PSUM"))
                psC = bctx.enter_context(tc.tile_pool(name="psC", bufs=2, space="PSUM"))

                for n in range(NCH):
                    qsb = qp.tile([P, 2 * NCW], bf16, name="qsb")
                    for kq in range(2):
                        qps = psQ.tile([P, NCW], f32, name=f"q{kq}")
                        for cc in range(4):
                            lo = cc * 256 + kq * 128
                            nc.tensor.matmul(
                                qps[:], c_wq[:, lo:lo + 128],
                                xall[:, cc * HW + n * NCW:
                                     cc * HW + (n + 1) * NCW],
                                start=(cc == 0), stop=(cc == 3))
                        nc.scalar.activation(qsb[:, kq * NCW:(kq + 1) * NCW],
                                             qps[:], AF.Relu,
                                             bias=c_bias[:, 6 + kq:7 + kq])
                    sT = stp.tile([S, NCW], bf16, name="sT")
                    for ns in range(4):
                        sps = psS.tile([P, S], f32, name="sim")
                        for kq in range(2):
                            nc.tensor.matmul(
                                sps[:],
                                qsb[:, kq * NCW + ns * 128:kq * NCW + (ns + 1) * 128],
                                keyn[:, kq * S:(kq + 1) * S],
                                start=(kq == 0), stop=(kq == 1))
                        pe = pp.tile([P, S], bf16, name="pe")
                        sums = sp.tile([P, 1], f32, name="sums")
                        nc.scalar.activation(pe[:], sps[:], AF.Exp,
                                             accum_out=sums[:])
                        rp = sp.tile([P, 1], f32, name="rp")
                        nc.vector.reciprocal(rp[:], sums[:])
                        pn = pp.tile([P, S], bf16, name="pn")
                        nc.vector.tensor_scalar_mul(pn[:], pe[:], rp[:])
                        tp2 = psT2.tile([P, 128], bf16, name="tp2")
                        nc.tensor.transpose(tp2[0:S, :], pn[:], c_id[:])
                        nc.scalar.copy(sT[:, ns * 128:(ns + 1) * 128], tp2[0:S, :])
                    outb = obp.tile([P, 4 * NCW], bf16, name="outb")
                    for cv in range(4):
                        cps = psC.tile([P, NCW], f32, name="ctx")
                        nc.tensor.matmul(cps[:], vT[:, cv * 128:(cv + 1) * 128],
                                         sT[:], start=True, stop=True)
                        nc.vector.tensor_add(outb[:, cv * NCW:(cv + 1) * NCW],
                                             cps[:],
                                             xall[:, cv * HW + n * NCW:
                                                  cv * HW + (n + 1) * NCW])
                    # store on the ScalarE HWDGE ring: keeps a resid-delayed
                    # store from head-of-line blocking the sync-ring x loads
                    nc.scalar.dma_start(
                        y_r[:, :, n * NCW:(n + 1) * NCW],
                        outb[:].rearrange("p (t n) -> p t n", t=4))

    nc.compile()
    return nc


def prep_host_inputs(inputs):
    """Fold BN affine into weights, build diag/scale/bias aux tensors."""
    g = lambda a: np.ascontiguousarray(np.asarray(a, dtype=np.float32))
    wq = (g(inputs["q_g"])[:, None] * g(inputs["q_w"])[:, :, 0, 0]).T
    wkp = (g(inputs["kp_g"])[:, None] * g(inputs["kp_w"])[:, :, 0, 0]).T
    wvp = (g(inputs["vp_g"])[:, None] * g(inputs["vp_w"])[:, :, 0, 0]).T
    wkc = g(inputs["kc_g"])[:, None] * g(inputs["kc_w"])[:, 0].reshape(128, 9)
    wvc = g(inputs["vc_g"])[:, None] * g(inputs["vc_w"])[:, 0].reshape(256, 9)

    diag = np.zeros((3, 9, 128, 128), np.float32)
    for t in range(9):
        diag[0, t] = np.diag(wkc[:, t])
        diag[1, t] = np.diag(wvc[:128, t])
        diag[2, t] = np.diag(wvc[128:, t])

    scale110 = np.zeros(S, np.float32)
    scale110[0] = 1.0 / 9216
    scale110[1:10] = 1.0 / 1024
    scale110[10:46] = 1.0 / 256
    scale110[46:110] = 1.0 / 144
    scl = np.zeros((2, 128, S), np.float32)
    scl[0] = scale110 / 16.0
    scl[1] = scale110

    bias = np.zeros((128, 8), np.float32)
    bias[:, 0] = g(inputs["kp_b"])
    bias[:, 1] = g(inputs["kc_b"])
    bias[:, 2] = g(inputs["vp_b"])[:128]
    bias[:, 3] = g(inputs["vp_b"])[128:]
    bias[:, 4] = g(inputs["vc_b"])[:128]
    bias[:, 5] = g(inputs["vc_b"])[128:]
    bias[:, 6] = g(inputs["q_b"])[:128]
    bias[:, 7] = g(inputs["q_b"])[128:]

    import ml_dtypes
    return {
        "wq": np.ascontiguousarray(wq).astype(ml_dtypes.bfloat16),
        "wkp": np.ascontiguousarray(wkp).astype(ml_dtypes.bfloat16),
        "wvp": np.ascontiguousarray(wvp).astype(ml_dtypes.bfloat16),
        "diag": diag.astype(ml_dtypes.bfloat16),
        "ident": np.eye(128, dtype=ml_dtypes.bfloat16),
        "scl": scl,
        "bias": bias,
    }


def make_in_maps(inputs):
    host = prep_host_inputs(inputs)
    x = np.asarray(inputs["x"], dtype=np.float32)
    B = x.shape[0]
    in_maps = []
    import ml_dtypes
    for b in range(B):
        m = dict(host)
        m["xb"] = np.ascontiguousarray(
            x[b].reshape(512, HW)).astype(ml_dtypes.bfloat16)
        in_maps.append(m)
    return in_maps


_NC = None


def get_nc():
    global _NC
    if _NC is None:
        _NC = build_bass()
    return _NC


def kernel(**inputs):
    from concourse import bass_utils
    nc = get_nc()
    in_maps = make_in_maps(inputs)
    res = bass_utils.run_bass_kernel_spmd(
        nc, in_maps, core_ids=list(range(len(in_maps))), trace=False)
    outs = [np.asarray(r["y"], dtype=np.float32).reshape(512, HH, HH)
            for r in res.results]
    return np.stack(outs, axis=0)



# revision 37
# speedup vs baseline: 1.5995x; 1.0715x over previous
"""CAPAttentionModule Trainium2 kernel.

Data-parallel over batch: 8 images -> 8 NeuronCores, one image per core.
Per core (x: [512, 9216] = [C, H*W], H=W=96):
  k1 = relu(Wkp x + b)              [128, HW]   (1x1 conv, BN folded)
  k2 = relu(dw3x3(k1) + b)          [128, HW]   (depthwise via diagonal matmuls)
  v1 = relu(Wvp x + b)              [256, HW]
  v2 = relu(dw3x3(v1) + b)          [256, HW]
  key = psp([k1;k2])   [256, 110],  value = psp([v1;v2])  [512, 110]
  q  = relu(Wq x + b)               [256, HW]
  sim = softmax_s(q^T key / 16)     [HW, 110]   (no max-subtract; |sim|<4)
  out = x + value @ sim^T           [512, HW]

All matmuls use float32r (full-rate fp32 on the PE at N>=256).
Depthwise 3x3 runs as 9 shifted diagonal matmuls accumulating in PSUM;
SAME-padding comes from a zero column pad (width 98 layout) plus
row-restricted APs at the image top/bottom (has_written overwrite
semantics make ragged accumulation exact).
PSP pooling: one 5D strided reduce to a 24x24 sum grid per map, then
small batched reduces for the 1/3/6/8 grids; normalization (and the
1/sqrt(256) sim scale) is folded into per-s scale tiles.
"""

import numpy as np

P = 128
HH = 96
WP = 98          # padded width/height (zero border ring)
HW = 9216
HWP = WP * WP    # 9604: [98, 98] with zero border, data at [1:97, 1:97]
RB = 24          # row blocks of 4 rows
RBN = 4 * HH     # 384
NCH = 18         # phase-B column chunks
NCW = 512
DWG = 6          # dw row-blocks per psum group
S = 110


def _f32r(ap):
    from concourse import mybir
    return ap.bitcast(mybir.dt.float32r)




def bass_ap_pool_view(ap_rows):
    """[p, >=4*WP] AP at the start of 4 data rows (stride WP) ->
    [p, wq, h, ws] view for a 4x4 pooling reduce over (h, ws)."""
    v = ap_rows[:, 0:4 * WP].rearrange("p (h w) -> p h w", w=WP)
    v = v[:, :, 0:HH]
    return v.rearrange("p h (wq ws) -> p wq h ws", ws=4)

def build_bass():
    import concourse.bacc as bacc
    import concourse.tile as tile
    from concourse import mybir
    from contextlib import ExitStack

    f32 = mybir.dt.float32
    f32r = mybir.dt.float32r
    bf16 = mybir.dt.bfloat16
    AF = mybir.ActivationFunctionType
    AX = mybir.AxisListType

    nc = bacc.Bacc("TRN2", target_bir_lowering=False, debug=False,
                   enable_asserts=False, num_devices=8)

    xb_d = nc.dram_tensor("xb", [512, HW], bf16, kind="ExternalInput").ap()
    wq_d = nc.dram_tensor("wq", [512, 256], bf16, kind="ExternalInput").ap()
    wkp_d = nc.dram_tensor("wkp", [512, 128], bf16, kind="ExternalInput").ap()
    wvp_d = nc.dram_tensor("wvp", [512, 256], bf16, kind="ExternalInput").ap()
    diag_d = nc.dram_tensor("diag", [3, 9, 128, 128], bf16, kind="ExternalInput").ap()
    id_d = nc.dram_tensor("ident", [128, 128], bf16, kind="ExternalInput").ap()
    scl_d = nc.dram_tensor("scl", [2, 128, S], f32, kind="ExternalInput").ap()
    bias_d = nc.dram_tensor("bias", [128, 8], f32, kind="ExternalInput").ap()
    y_d = nc.dram_tensor("y", [512, HW], bf16, kind="ExternalOutput").ap()

    xb_r = xb_d.rearrange("(t p) n -> p t n", p=P)
    y_r = y_d.rearrange("(t p) n -> p t n", p=P)

    with tile.TileContext(nc) as tc:
        with ExitStack() as top:
            cpool = top.enter_context(tc.tile_pool(name="consts", bufs=1))
            kpool = top.enter_context(tc.tile_pool(name="keep", bufs=1))

            c_wkp = cpool.tile([P, 4 * 128], bf16)
            nc.sync.dma_start(c_wkp[:].rearrange("p (t m) -> p t m", t=4),
                              wkp_d.rearrange("(t p) m -> p t m", p=P))
            c_wvp = cpool.tile([P, 4 * 256], bf16)
            nc.sync.dma_start(c_wvp[:].rearrange("p (t m) -> p t m", t=4),
                              wvp_d.rearrange("(t p) m -> p t m", p=P))
            c_bias = cpool.tile([P, 8], f32)
            nc.sync.dma_start(c_bias[:], bias_d)
            c_wq = cpool.tile([P, 4 * 256], bf16)
            nc.scalar.dma_start(c_wq[:].rearrange("p (t m) -> p t m", t=4),
                                wq_d.rearrange("(t p) m -> p t m", p=P))
            c_dg = cpool.tile([P, 27 * 128], bf16)
            nc.scalar.dma_start(c_dg[:].rearrange("p (ct m) -> p ct m", ct=27),
                                diag_d.rearrange("c t p m -> p (c t) m"))
            c_id = cpool.tile([P, 128], bf16)
            nc.scalar.dma_start(c_id[:], id_d)
            c_scl = cpool.tile([P, 2 * S], f32)
            nc.scalar.dma_start(c_scl[:].rearrange("p (s m) -> p s m", s=2),
                                scl_d.rearrange("s p m -> p s m"))

            keyn = kpool.tile([P, 2 * S], bf16)       # normalized key (incl /16)
            vT = kpool.tile([S, 512], bf16)           # value^T [s, c]
            # x (bf16) resident in SBUF for both phases: [p, (t, n)]
            xall = kpool.tile([P, 4 * HW], bf16)
            xall_v = xall[:].rearrange("p (t n) -> p t n", t=4)

            # ---------------- Phase A: key/value branches ----------------
            with ExitStack() as actx:
                bigp = actx.enter_context(tc.tile_pool(name="bigA", bufs=1))
                blkp = actx.enter_context(tc.tile_pool(name="blk", bufs=6))
                tmpp = actx.enter_context(tc.tile_pool(name="tmpA", bufs=1))

                k1p = bigp.tile([P, HWP], bf16)
                v1p = bigp.tile([P, 2 * HWP], bf16)
                p24 = bigp.tile([P, 6 * 576], f32)
                allp = bigp.tile([P, 6 * S], f32)
                valn = bigp.tile([P, 4 * S], bf16)

                # zero the pad border (rows 0/97, cols 0/97)
                for chv in (k1p[:, 0:HWP], v1p[:, 0:HWP], v1p[:, HWP:2 * HWP]):
                    c3 = chv.rearrange("p (h w) -> p h w", w=WP)
                    nc.gpsimd.memset(c3[:, 0:1, :], 0.0)
                    nc.gpsimd.memset(c3[:, 97:98, :], 0.0)
                    nc.gpsimd.memset(c3[:, 1:97, 0:1], 0.0)
                    nc.gpsimd.memset(c3[:, 1:97, 97:98], 0.0)

                # primary 1x1 convs, streamed by 4-row blocks (2 blocks/DMA),
                # with per-block pooling of k1/v1a/v1b interleaved on DVE
                with tc.tile_pool(name="psA", bufs=2, space="PSUM") as psA:
                    for rbb in range(RB // 2):
                        nc.sync.dma_start(
                            xall_v[:, :, rbb * 2 * RBN:(rbb + 1) * 2 * RBN],
                            xb_r[:, :, rbb * 2 * RBN:(rbb + 1) * 2 * RBN])
                        dsts = [
                            (k1p, 0, c_wkp, 128, 0, 0),
                            (v1p, 0, c_wvp, 256, 2, 2),
                            (v1p, 1, c_wvp, 256, 3, 3),
                        ]
                        for sub in range(2):
                            rb = rbb * 2 + sub
                            for di, (dst, half, wt, wm, bcol, slot) in enumerate(dsts):
                                ps = psA.tile([P, RBN], f32, name=f"pps{di}")
                                for cc in range(4):
                                    lo = cc * wm + (half * 128 if wm == 256 else 0)
                                    nc.tensor.matmul(
                                        ps[:], wt[:, lo:lo + 128],
                                        xall[:, cc * HW + rb * RBN:
                                             cc * HW + (rb + 1) * RBN],
                                        start=(cc == 0), stop=(cc == 3))
                                dv = dst[:, half * HWP:(half + 1) * HWP].rearrange(
                                    "p (h w) -> p h w", w=WP)
                                nc.scalar.activation(
                                    dv[:, 4 * rb + 1:4 * rb + 5, 1:97],
                                    ps[:].rearrange("p (h w) -> p h w", w=HH),
                                    AF.Relu, bias=c_bias[:, bcol:bcol + 1])
                                st = (4 * rb + 1) * WP + 1
                                pv = dst[:, half * HWP + st:half * HWP + st + 4 * WP]
                                pv = bass_ap_pool_view(pv)
                                nc.vector.reduce_sum(
                                    p24[:, slot * 576 + rb * 24:slot * 576 + (rb + 1) * 24],
                                    pv, axis=AX.XY)

                # small pools over a map range [m0, m1) -> allp columns
                def smallpools(m0, m1):
                    m = m1 - m0
                    allp_v = allp[:, m0 * S:m1 * S].rearrange(
                        "p (m s) -> p m s", s=S)
                    p24s = p24[:, m0 * 576:m1 * 576]
                    nc.vector.reduce_sum(
                        allp_v[:, :, 0:1],
                        p24s.rearrange("p (m s) -> p m s", s=576), axis=AX.X)
                    tmp = tmpp.tile([P, 1152], f32, name="tmp", tag="tmp")
                    nc.vector.reduce_sum(
                        tmp[:, 0:m * 72],
                        p24s.rearrange("p (mh wq ws) -> p mh wq ws", wq=3, ws=8),
                        axis=AX.X)
                    nc.vector.reduce_sum(
                        allp_v[:, :, 1:10],
                        tmp[:, 0:m * 72].rearrange(
                            "p (m hq hs wq) -> p m hq wq hs", m=m, hq=3, hs=8),
                        axis=AX.X)
                    tmp6 = tmpp.tile([P, 1152], f32, name="tmp6", tag="tmp")
                    nc.vector.reduce_sum(
                        tmp6[:, 0:m * 144],
                        p24s.rearrange("p (mh wq ws) -> p mh wq ws", wq=6, ws=4),
                        axis=AX.X)
                    nc.vector.reduce_sum(
                        allp_v[:, :, 10:46],
                        tmp6[:, 0:m * 144].rearrange(
                            "p (m hq hs wq) -> p m hq wq hs", m=m, hq=6, hs=4),
                        axis=AX.X)
                    tmp8 = tmpp.tile([P, 1152], f32, name="tmp8", tag="tmp")
                    nc.vector.reduce_sum(
                        tmp8[:, 0:m * 192],
                        p24s.rearrange("p (mh wq ws) -> p mh wq ws", wq=8, ws=3),
                        axis=AX.X)
                    nc.vector.reduce_sum(
                        allp_v[:, :, 46:110],
                        tmp8[:, 0:m * 192].rearrange(
                            "p (m hq hs wq) -> p m hq wq hs", m=m, hq=8, hs=3),
                        axis=AX.X)


                # depthwise 3x3 via diagonal matmuls + pooling of k2/v2;
                # value maps pooled/transposed as soon as each is complete
                def vt_build(j):
                    tp = psTp.tile([P, 128], bf16, name="tp", tag="tp")
                    nc.tensor.transpose(tp[0:S, :], valn[:, j * S:(j + 1) * S],
                                        c_id[:])
                    nc.scalar.copy(vT[:, j * 128:(j + 1) * 128], tp[0:S, :])

                def val_finish(m0, m1):
                    smallpools(m0, m1)
                    for mm in range(m0, m1):
                        j = mm - 2
                        nc.vector.tensor_mul(valn[:, j * S:(j + 1) * S],
                                             allp[:, mm * S:(mm + 1) * S],
                                             c_scl[:, S:2 * S])
                        vt_build(j)

                with tc.tile_pool(name="psD", bufs=1, space="PSUM") as psD, \
                        tc.tile_pool(name="psTa", bufs=2, space="PSUM") as psTp:
                    # maps 2,3 (v1a, v1b) complete after the primary loop
                    val_finish(2, 4)
                    chunks = [(k1p[:, 0:HWP], 0, 1, 1),
                              (v1p[:, 0:HWP], 1, 4, 4),
                              (v1p[:, HWP:2 * HWP], 2, 5, 5)]
                    for chv, ci, bcol, slot in chunks:
                        ch3 = chv.rearrange("p (h w) -> p h w", w=WP)
                        for g in range(RB // DWG):
                            pss = [psD.tile([P, RBN], f32, name=f"dw{j}")
                                   for j in range(DWG)]
                            for t in range(9):
                                dy, dx = t // 3, t % 3
                                dgap = c_dg[:, (ci * 9 + t) * 128:(ci * 9 + t + 1) * 128]
                                for j in range(DWG):
                                    r0 = (g * DWG + j) * 4
                                    rhs = ch3[:, r0 + dy:r0 + dy + 4, dx:dx + HH]
                                    nc.tensor.matmul(
                                        pss[j][:], dgap, rhs,
                                        start=(t == 0), stop=(t == 8))
                            for j in range(DWG):
                                rb = g * DWG + j
                                blk = blkp.tile([P, RBN], bf16, name="blk")
                                nc.scalar.activation(
                                    blk[:], pss[j][:], AF.Relu,
                                    bias=c_bias[:, bcol:bcol + 1])
                                bv = blk[:].rearrange(
                                    "p (h wq ws) -> p wq h ws", h=4, ws=4)
                                nc.vector.reduce_sum(
                                    p24[:, slot * 576 + rb * 24:slot * 576 + (rb + 1) * 24],
                                    bv, axis=AX.XY)
                        if ci == 0:
                            # key branch done: pool + normalize immediately so
                            # phase-B sim/softmax can overlap the value chunks
                            smallpools(0, 2)
                            for kq in range(2):
                                nc.vector.tensor_mul(
                                    keyn[:, kq * S:(kq + 1) * S],
                                    allp[:, kq * S:(kq + 1) * S], c_scl[:, 0:S])
                        elif ci == 1:
                            val_finish(4, 5)
                        else:
                            val_finish(5, 6)


            # ---------------- Phase B: query / attention / output ----------------
            with ExitStack() as bctx:
                qp = bctx.enter_context(tc.tile_pool(name="qsb", bufs=5))
                pp = bctx.enter_context(tc.tile_pool(name="pexp", bufs=8))
                sp = bctx.enter_context(tc.tile_pool(name="small", bufs=8))
                stp = bctx.enter_context(tc.tile_pool(name="simT", bufs=5))
                obp = bctx.enter_context(tc.tile_pool(name="outb", bufs=3))
                psQ = bctx.enter_context(tc.tile_pool(name="psQ", bufs=1, space="PSUM"))
                psS = bctx.enter_context(tc.tile_pool(name="psS", bufs=2, space="PSUM"))
                psT2 = bctx.enter_context(tc.tile_pool(name="psT2", bufs=2, space="PSUM"))
                psC = bctx.enter_context(tc.tile_pool(name="psC", bufs=2, space="PSUM"))

                for n in range(NCH):
                    qsb = qp.tile([P, 2 * NCW], bf16, name="qsb")
                    for kq in range(2):
                        qps = psQ.tile([P, NCW], f32, name=f"q{kq}")
                        for cc in range(4):
                            lo = cc * 256 + kq * 128
                            nc.tensor.matmul(
                                qps[:], c_wq[:, lo:lo + 128],
                                xall[:, cc * HW + n * NCW:
                                     cc * HW + (n + 1) * NCW],
                                start=(cc == 0), stop=(cc == 3))
                        nc.scalar.activation(qsb[:, kq * NCW:(kq + 1) * NCW],
                                             qps[:], AF.Relu,
                                             bias=c_bias[:, 6 + kq:7 + kq])
                    sT = stp.tile([S, NCW], bf16, name="sT")
                    for ns in range(4):
                        sps = psS.tile([P, S], f32, name="sim")
                        for kq in range(2):
                            nc.tensor.matmul(
                                sps[:],
                                qsb[:, kq * NCW + ns * 128:kq * NCW + (ns + 1) * 128],
                                keyn[:, kq * S:(kq + 1) * S],
                                start=(kq == 0), stop=(kq == 1))
                        pe = pp.tile([P, S], bf16, name="pe")
                        sums = sp.tile([P, 1], f32, name="sums")
                        nc.scalar.activation(pe[:], sps[:], AF.Exp)
                        nc.vector.reduce_sum(sums[:], pe[:], axis=AX.X)
                        rp = sp.tile([P, 1], f32, name="rp")
                        nc.vector.reciprocal(rp[:], sums[:])
                        pn = pp.tile([P, S], bf16, name="pn")
                        nc.vector.tensor_scalar_mul(pn[:], pe[:], rp[:])
                        tp2 = psT2.tile([P, 128], bf16, name="tp2")
                        nc.tensor.transpose(tp2[0:S, :], pn[:], c_id[:])
                        nc.scalar.copy(sT[:, ns * 128:(ns + 1) * 128], tp2[0:S, :])
                    outb = obp.tile([P, 4 * NCW], bf16, name="outb")
                    for cv in range(4):
                        cps = psC.tile([P, NCW], f32, name="ctx")
                        nc.tensor.matmul(cps[:], vT[:, cv * 128:(cv + 1) * 128],
                                         sT[:], start=True, stop=True)
                        nc.vector.tensor_add(outb[:, cv * NCW:(cv + 1) * NCW],
                                             cps[:],
                                             xall[:, cv * HW + n * NCW:
                                                  cv * HW + (n + 1) * NCW])
                    # store on the ScalarE HWDGE ring: keeps a resid-delayed
                    # store from head-of-line blocking the sync-ring x loads
                    nc.scalar.dma_start(
                        y_r[:, :, n * NCW:(n + 1) * NCW],
                        outb[:].rearrange("p (t n) -> p t n", t=4))

    nc.compile()
    return nc


def prep_host_inputs(inputs):
    """Fold BN affine into weights, build diag/scale/bias aux tensors."""
    g = lambda a: np.ascontiguousarray(np.asarray(a, dtype=np.float32))
    wq = (g(inputs["q_g"])[:, None] * g(inputs["q_w"])[:, :, 0, 0]).T
    wkp = (g(inputs["kp_g"])[:, None] * g(inputs["kp_w"])[:, :, 0, 0]).T
    wvp = (g(inputs["vp_g"])[:, None] * g(inputs["vp_w"])[:, :, 0, 0]).T
    wkc = g(inputs["kc_g"])[:, None] * g(inputs["kc_w"])[:, 0].reshape(128, 9)
    wvc = g(inputs["vc_g"])[:, None] * g(inputs["vc_w"])[:, 0].reshape(256, 9)

    diag = np.zeros((3, 9, 128, 128), np.float32)
    for t in range(9):
        diag[0, t] = np.diag(wkc[:, t])
        diag[1, t] = np.diag(wvc[:128, t])
        diag[2, t] = np.diag(wvc[128:, t])

    scale110 = np.zeros(S, np.float32)
    scale110[0] = 1.0 / 9216
    scale110[1:10] = 1.0 / 1024
    scale110[10:46] = 1.0 / 256
    scale110[46:110] = 1.0 / 144
    scl = np.zeros((2, 128, S), np.float32)
    scl[0] = scale110 / 16.0
    scl[1] = scale110

    bias = np.zeros((128, 8), np.float32)
    bias[:, 0] = g(inputs["kp_b"])
    bias[:, 1] = g(inputs["kc_b"])
    bias[:, 2] = g(inputs["vp_b"])[:128]
    bias[:, 3] = g(inputs["vp_b"])[128:]
    bias[:, 4] = g(inputs["vc_b"])[:128]
    bias[:, 5] = g(inputs["vc_b"])[128:]
    bias[:, 6] = g(inputs["q_b"])[:128]
    bias[:, 7] = g(inputs["q_b"])[128:]

    import ml_dtypes
    return {
        "wq": np.ascontiguousarray(wq).astype(ml_dtypes.bfloat16),
        "wkp": np.ascontiguousarray(wkp).astype(ml_dtypes.bfloat16),
        "wvp": np.ascontiguousarray(wvp).astype(ml_dtypes.bfloat16),
        "diag": diag.astype(ml_dtypes.bfloat16),
        "ident": np.eye(128, dtype=ml_dtypes.bfloat16),
        "scl": scl,
        "bias": bias,
    }


def make_in_maps(inputs):
    host = prep_host_inputs(inputs)
    x = np.asarray(inputs["x"], dtype=np.float32)
    B = x.shape[0]
    in_maps = []
    import ml_dtypes
    for b in range(B):
        m = dict(host)
        m["xb"] = np.ascontiguousarray(
            x[b].reshape(512, HW)).astype(ml_dtypes.bfloat16)
        in_maps.append(m)
    return in_maps


_NC = None


def get_nc():
    global _NC
    if _NC is None:
        _NC = build_bass()
    return _NC


def kernel(**inputs):
    from concourse import bass_utils
    nc = get_nc()
    in_maps = make_in_maps(inputs)
    res = bass_utils.run_bass_kernel_spmd(
        nc, in_maps, core_ids=list(range(len(in_maps))), trace=False)
    outs = [np.asarray(r["y"], dtype=np.float32).reshape(512, HH, HH)
            for r in res.results]
    return np.stack(outs, axis=0)



# revision 40
# speedup vs baseline: 1.6463x; 1.0293x over previous
"""CAPAttentionModule Trainium2 kernel.

Data-parallel over batch: 8 images -> 8 NeuronCores, one image per core.
Per core (x: [512, 9216] = [C, H*W], H=W=96):
  k1 = relu(Wkp x + b)              [128, HW]   (1x1 conv, BN folded)
  k2 = relu(dw3x3(k1) + b)          [128, HW]   (depthwise via diagonal matmuls)
  v1 = relu(Wvp x + b)              [256, HW]
  v2 = relu(dw3x3(v1) + b)          [256, HW]
  key = psp([k1;k2])   [256, 110],  value = psp([v1;v2])  [512, 110]
  q  = relu(Wq x + b)               [256, HW]
  sim = softmax_s(q^T key / 16)     [HW, 110]   (no max-subtract; |sim|<4)
  out = x + value @ sim^T           [512, HW]

All matmuls use float32r (full-rate fp32 on the PE at N>=256).
Depthwise 3x3 runs as 9 shifted diagonal matmuls accumulating in PSUM;
SAME-padding comes from a zero column pad (width 98 layout) plus
row-restricted APs at the image top/bottom (has_written overwrite
semantics make ragged accumulation exact).
PSP pooling: one 5D strided reduce to a 24x24 sum grid per map, then
small batched reduces for the 1/3/6/8 grids; normalization (and the
1/sqrt(256) sim scale) is folded into per-s scale tiles.
"""

import numpy as np

P = 128
HH = 96
WP = 98          # padded width/height (zero border ring)
HW = 9216
HWP = WP * WP    # 9604: [98, 98] with zero border, data at [1:97, 1:97]
RB = 24          # row blocks of 4 rows
RBN = 4 * HH     # 384
NCH = 18         # phase-B column chunks
NCW = 512
DWG = 6          # dw row-blocks per psum group
S = 110


def _f32r(ap):
    from concourse import mybir
    return ap.bitcast(mybir.dt.float32r)




def bass_ap_pool_view(ap_rows):
    """[p, >=4*WP] AP at the start of 4 data rows (stride WP) ->
    [p, wq, h, ws] view for a 4x4 pooling reduce over (h, ws)."""
    v = ap_rows[:, 0:4 * WP].rearrange("p (h w) -> p h w", w=WP)
    v = v[:, :, 0:HH]
    return v.rearrange("p h (wq ws) -> p wq h ws", ws=4)

def build_bass():
    import concourse.bacc as bacc
    import concourse.tile as tile
    from concourse import mybir
    from contextlib import ExitStack

    f32 = mybir.dt.float32
    f32r = mybir.dt.float32r
    bf16 = mybir.dt.bfloat16
    AF = mybir.ActivationFunctionType
    AX = mybir.AxisListType

    nc = bacc.Bacc("TRN2", target_bir_lowering=False, debug=False,
                   enable_asserts=False, num_devices=8)

    f8 = mybir.dt.float8e4
    MPM = mybir.MatmulPerfMode
    xb_d = nc.dram_tensor("xb", [512, HW], bf16, kind="ExternalInput").ap()
    xf8_d = nc.dram_tensor("xf8", [512, HW], f8, kind="ExternalInput").ap()
    wkp8_d = nc.dram_tensor("wkp8", [512, 128], f8, kind="ExternalInput").ap()
    wvp8_d = nc.dram_tensor("wvp8", [512, 256], f8, kind="ExternalInput").ap()
    wq_d = nc.dram_tensor("wq", [512, 256], bf16, kind="ExternalInput").ap()
    wkp_d = nc.dram_tensor("wkp", [512, 128], bf16, kind="ExternalInput").ap()
    wvp_d = nc.dram_tensor("wvp", [512, 256], bf16, kind="ExternalInput").ap()
    diag_d = nc.dram_tensor("diag", [3, 9, 128, 128], bf16, kind="ExternalInput").ap()
    id_d = nc.dram_tensor("ident", [128, 128], bf16, kind="ExternalInput").ap()
    scl_d = nc.dram_tensor("scl", [2, 128, S], f32, kind="ExternalInput").ap()
    bias_d = nc.dram_tensor("bias", [128, 8], f32, kind="ExternalInput").ap()
    y_d = nc.dram_tensor("y", [512, HW], bf16, kind="ExternalOutput").ap()

    xb_r = xb_d.rearrange("(t p) n -> p t n", p=P)
    xf8_r = xf8_d.rearrange("(t p) n -> p t n", p=P)
    y_r = y_d.rearrange("(t p) n -> p t n", p=P)

    with tile.TileContext(nc) as tc:
        with ExitStack() as top:
            cpool = top.enter_context(tc.tile_pool(name="consts", bufs=1))
            kpool = top.enter_context(tc.tile_pool(name="keep", bufs=1))

            c_wkp = cpool.tile([P, 4 * 128], f8)
            nc.sync.dma_start(c_wkp[:].rearrange("p (t m) -> p t m", t=4),
                              wkp8_d.rearrange("(t p) m -> p t m", p=P))
            c_wvp = cpool.tile([P, 4 * 256], f8)
            nc.sync.dma_start(c_wvp[:].rearrange("p (t m) -> p t m", t=4),
                              wvp8_d.rearrange("(t p) m -> p t m", p=P))
            c_bias = cpool.tile([P, 8], f32)
            nc.sync.dma_start(c_bias[:], bias_d)
            c_wq = cpool.tile([P, 4 * 256], bf16)
            nc.scalar.dma_start(c_wq[:].rearrange("p (t m) -> p t m", t=4),
                                wq_d.rearrange("(t p) m -> p t m", p=P))
            c_dg = cpool.tile([P, 27 * 128], bf16)
            nc.scalar.dma_start(c_dg[:].rearrange("p (ct m) -> p ct m", ct=27),
                                diag_d.rearrange("c t p m -> p (c t) m"))
            c_id = cpool.tile([P, 128], bf16)
            nc.scalar.dma_start(c_id[:], id_d)
            c_scl = cpool.tile([P, 2 * S], f32)
            nc.scalar.dma_start(c_scl[:].rearrange("p (s m) -> p s m", s=2),
                                scl_d.rearrange("s p m -> p s m"))

            keyn = kpool.tile([P, 2 * S], bf16)       # normalized key (incl /16)
            vT = kpool.tile([S, 512], bf16)           # value^T [s, c]
            # x (bf16) resident in SBUF for both phases: [p, (t, n)]
            xall = kpool.tile([P, 4 * HW], bf16)
            xall_v = xall[:].rearrange("p (t n) -> p t n", t=4)

            # ---------------- Phase A: key/value branches ----------------
            with ExitStack() as actx:
                bigp = actx.enter_context(tc.tile_pool(name="bigA", bufs=1))
                x8p = actx.enter_context(tc.tile_pool(name="x8", bufs=3))
                blkp = actx.enter_context(tc.tile_pool(name="blk", bufs=6))
                tmpp = actx.enter_context(tc.tile_pool(name="tmpA", bufs=1))

                k1p = bigp.tile([P, HWP], bf16)
                v1p = bigp.tile([P, 2 * HWP], bf16)
                p24 = bigp.tile([P, 6 * 576], f32)
                allp = bigp.tile([P, 6 * S], f32)
                valn = bigp.tile([P, 4 * S], bf16)

                # zero the pad border (rows 0/97, cols 0/97)
                for chv in (k1p[:, 0:HWP], v1p[:, 0:HWP], v1p[:, HWP:2 * HWP]):
                    c3 = chv.rearrange("p (h w) -> p h w", w=WP)
                    nc.gpsimd.memset(c3[:, 0:1, :], 0.0)
                    nc.gpsimd.memset(c3[:, 97:98, :], 0.0)
                    nc.gpsimd.memset(c3[:, 1:97, 0:1], 0.0)
                    nc.gpsimd.memset(c3[:, 1:97, 97:98], 0.0)

                # primary 1x1 convs, streamed by 4-row blocks (2 blocks/DMA),
                # with per-block pooling of k1/v1a/v1b interleaved on DVE
                with tc.tile_pool(name="psA", bufs=2, space="PSUM") as psA:
                    for rbb in range(RB // 2):
                        nc.sync.dma_start(
                            xall_v[:, :, rbb * 2 * RBN:(rbb + 1) * 2 * RBN],
                            xb_r[:, :, rbb * 2 * RBN:(rbb + 1) * 2 * RBN])
                        xt8 = x8p.tile([P, 4 * 2 * RBN], f8, name="xt8")
                        nc.sync.dma_start(
                            xt8[:].rearrange("p (t n) -> p t n", t=4),
                            xf8_r[:, :, rbb * 2 * RBN:(rbb + 1) * 2 * RBN])
                        xt8_v = xt8[:].rearrange("p (t n) -> p t n", t=4)
                        dsts = [
                            (k1p, 0, c_wkp, 128, 0, 0),
                            (v1p, 0, c_wvp, 256, 2, 2),
                            (v1p, 1, c_wvp, 256, 3, 3),
                        ]
                        for sub in range(2):
                            rb = rbb * 2 + sub
                            for di, (dst, half, wt, wm, bcol, slot) in enumerate(dsts):
                                ps = psA.tile([P, RBN], f32, name=f"pps{di}")
                                wtv = wt[:].rearrange("p (t m) -> p t m", t=4)
                                off = half * 128 if wm == 256 else 0
                                for pr in range(2):
                                    nc.tensor.matmul(
                                        ps[:],
                                        wtv[:, 2 * pr:2 * pr + 2,
                                            off:off + 128],
                                        xt8_v[:, 2 * pr:2 * pr + 2,
                                              sub * RBN:(sub + 1) * RBN],
                                        start=(pr == 0), stop=(pr == 1),
                                        perf_mode=MPM.DoubleRow)
                                dv = dst[:, half * HWP:(half + 1) * HWP].rearrange(
                                    "p (h w) -> p h w", w=WP)
                                nc.scalar.activation(
                                    dv[:, 4 * rb + 1:4 * rb + 5, 1:97],
                                    ps[:].rearrange("p (h w) -> p h w", w=HH),
                                    AF.Relu, bias=c_bias[:, bcol:bcol + 1])
                                st = (4 * rb + 1) * WP + 1
                                pv = dst[:, half * HWP + st:half * HWP + st + 4 * WP]
                                pv = bass_ap_pool_view(pv)
                                nc.vector.reduce_sum(
                                    p24[:, slot * 576 + rb * 24:slot * 576 + (rb + 1) * 24],
                                    pv, axis=AX.XY)

                # small pools over a map range [m0, m1) -> allp columns
                def smallpools(m0, m1):
                    m = m1 - m0
                    allp_v = allp[:, m0 * S:m1 * S].rearrange(
                        "p (m s) -> p m s", s=S)
                    p24s = p24[:, m0 * 576:m1 * 576]
                    nc.vector.reduce_sum(
                        allp_v[:, :, 0:1],
                        p24s.rearrange("p (m s) -> p m s", s=576), axis=AX.X)
                    tmp = tmpp.tile([P, 1152], f32, name="tmp", tag="tmp")
                    nc.vector.reduce_sum(
                        tmp[:, 0:m * 72],
                        p24s.rearrange("p (mh wq ws) -> p mh wq ws", wq=3, ws=8),
                        axis=AX.X)
                    nc.vector.reduce_sum(
                        allp_v[:, :, 1:10],
                        tmp[:, 0:m * 72].rearrange(
                            "p (m hq hs wq) -> p m hq wq hs", m=m, hq=3, hs=8),
                        axis=AX.X)
                    tmp6 = tmpp.tile([P, 1152], f32, name="tmp6", tag="tmp")
                    nc.vector.reduce_sum(
                        tmp6[:, 0:m * 144],
                        p24s.rearrange("p (mh wq ws) -> p mh wq ws", wq=6, ws=4),
                        axis=AX.X)
                    nc.vector.reduce_sum(
                        allp_v[:, :, 10:46],
                        tmp6[:, 0:m * 144].rearrange(
                            "p (m hq hs wq) -> p m hq wq hs", m=m, hq=6, hs=4),
                        axis=AX.X)
                    tmp8 = tmpp.tile([P, 1152], f32, name="tmp8", tag="tmp")
                    nc.vector.reduce_sum(
                        tmp8[:, 0:m * 192],
                        p24s.rearrange("p (mh wq ws) -> p mh wq ws", wq=8, ws=3),
                        axis=AX.X)
                    nc.vector.reduce_sum(
                        allp_v[:, :, 46:110],
                        tmp8[:, 0:m * 192].rearrange(
                            "p (m hq hs wq) -> p m hq wq hs", m=m, hq=8, hs=3),
                        axis=AX.X)


                # depthwise 3x3 via diagonal matmuls + pooling of k2/v2;
                # value maps pooled/transposed as soon as each is complete
                def vt_build(j):
                    tp = psTp.tile([P, 128], bf16, name="tp", tag="tp")
                    nc.tensor.transpose(tp[0:S, :], valn[:, j * S:(j + 1) * S],
                                        c_id[:])
                    nc.scalar.copy(vT[:, j * 128:(j + 1) * 128], tp[0:S, :])

                def val_finish(m0, m1):
                    smallpools(m0, m1)
                    for mm in range(m0, m1):
                        j = mm - 2
                        nc.vector.tensor_mul(valn[:, j * S:(j + 1) * S],
                                             allp[:, mm * S:(mm + 1) * S],
                                             c_scl[:, S:2 * S])
                        vt_build(j)

                with tc.tile_pool(name="psD", bufs=1, space="PSUM") as psD, \
                        tc.tile_pool(name="psTa", bufs=2, space="PSUM") as psTp:
                    # maps 2,3 (v1a, v1b) complete after the primary loop
                    val_finish(2, 4)
                    chunks = [(k1p[:, 0:HWP], 0, 1, 1),
                              (v1p[:, 0:HWP], 1, 4, 4),
                              (v1p[:, HWP:2 * HWP], 2, 5, 5)]
                    for chv, ci, bcol, slot in chunks:
                        ch3 = chv.rearrange("p (h w) -> p h w", w=WP)
                        for g in range(RB // DWG):
                            pss = [psD.tile([P, RBN], f32, name=f"dw{j}")
                                   for j in range(DWG)]
                            for t in range(9):
                                dy, dx = t // 3, t % 3
                                dgap = c_dg[:, (ci * 9 + t) * 128:(ci * 9 + t + 1) * 128]
                                for j in range(DWG):
                                    r0 = (g * DWG + j) * 4
                                    rhs = ch3[:, r0 + dy:r0 + dy + 4, dx:dx + HH]
                                    nc.tensor.matmul(
                                        pss[j][:], dgap, rhs,
                                        start=(t == 0), stop=(t == 8))
                            for j in range(DWG):
                                rb = g * DWG + j
                                blk = blkp.tile([P, RBN], bf16, name="blk")
                                nc.scalar.activation(
                                    blk[:], pss[j][:], AF.Relu,
                                    bias=c_bias[:, bcol:bcol + 1])
                                bv = blk[:].rearrange(
                                    "p (h wq ws) -> p wq h ws", h=4, ws=4)
                                nc.vector.reduce_sum(
                                    p24[:, slot * 576 + rb * 24:slot * 576 + (rb + 1) * 24],
                                    bv, axis=AX.XY)
                        if ci == 0:
                            # key branch done: pool + normalize immediately so
                            # phase-B sim/softmax can overlap the value chunks
                            smallpools(0, 2)
                            for kq in range(2):
                                nc.vector.tensor_mul(
                                    keyn[:, kq * S:(kq + 1) * S],
                                    allp[:, kq * S:(kq + 1) * S], c_scl[:, 0:S])
                        elif ci == 1:
                            val_finish(4, 5)
                        else:
                            val_finish(5, 6)


            # ---------------- Phase B: query / attention / output ----------------
            with ExitStack() as bctx:
                qp = bctx.enter_context(tc.tile_pool(name="qsb", bufs=5))
                pp = bctx.enter_context(tc.tile_pool(name="pexp", bufs=8))
                sp = bctx.enter_context(tc.tile_pool(name="small", bufs=8))
                stp = bctx.enter_context(tc.tile_pool(name="simT", bufs=5))
                obp = bctx.enter_context(tc.tile_pool(name="outb", bufs=3))
                psQ = bctx.enter_context(tc.tile_pool(name="psQ", bufs=1, space="PSUM"))
                psS = bctx.enter_context(tc.tile_pool(name="psS", bufs=2, space="PSUM"))
                psT2 = bctx.enter_context(tc.tile_pool(name="psT2", bufs=2, space="PSUM"))
                psC = bctx.enter_context(tc.tile_pool(name="psC", bufs=2, space="PSUM"))

                for n in range(NCH):
                    qsb = qp.tile([P, 2 * NCW], bf16, name="qsb")
                    for kq in range(2):
                        qps = psQ.tile([P, NCW], f32, name=f"q{kq}")
                        for cc in range(4):
                            lo = cc * 256 + kq * 128
                            nc.tensor.matmul(
                                qps[:], c_wq[:, lo:lo + 128],
                                xall[:, cc * HW + n * NCW:
                                     cc * HW + (n + 1) * NCW],
                                start=(cc == 0), stop=(cc == 3))
                        nc.scalar.activation(qsb[:, kq * NCW:(kq + 1) * NCW],
                                             qps[:], AF.Relu,
                                             bias=c_bias[:, 6 + kq:7 + kq])
                    sT = stp.tile([S, NCW], bf16, name="sT")
                    for ns in range(4):
                        sps = psS.tile([P, S], f32, name="sim")
                        for kq in range(2):
                            nc.tensor.matmul(
                                sps[:],
                                qsb[:, kq * NCW + ns * 128:kq * NCW + (ns + 1) * 128],
                                keyn[:, kq * S:(kq + 1) * S],
                                start=(kq == 0), stop=(kq == 1))
                        pe = pp.tile([P, S], bf16, name="pe")
                        sums = sp.tile([P, 1], f32, name="sums")
                        nc.scalar.activation(pe[:], sps[:], AF.Exp)
                        nc.vector.reduce_sum(sums[:], pe[:], axis=AX.X)
                        rp = sp.tile([P, 1], f32, name="rp")
                        nc.vector.reciprocal(rp[:], sums[:])
                        pn = pp.tile([P, S], bf16, name="pn")
                        nc.vector.tensor_scalar_mul(pn[:], pe[:], rp[:])
                        tp2 = psT2.tile([P, 128], bf16, name="tp2")
                        nc.tensor.transpose(tp2[0:S, :], pn[:], c_id[:])
                        nc.scalar.copy(sT[:, ns * 128:(ns + 1) * 128], tp2[0:S, :])
                    outb = obp.tile([P, 4 * NCW], bf16, name="outb")
                    for cv in range(4):
                        cps = psC.tile([P, NCW], f32, name="ctx")
                        nc.tensor.matmul(cps[:], vT[:, cv * 128:(cv + 1) * 128],
                                         sT[:], start=True, stop=True)
                        nc.vector.tensor_add(outb[:, cv * NCW:(cv + 1) * NCW],
                                             cps[:],
                                             xall[:, cv * HW + n * NCW:
                                                  cv * HW + (n + 1) * NCW])
                    # store on the ScalarE HWDGE ring: keeps a resid-delayed
                    # store from head-of-line blocking the sync-ring x loads
                    nc.scalar.dma_start(
                        y_r[:, :, n * NCW:(n + 1) * NCW],
                        outb[:].rearrange("p (t n) -> p t n", t=4))

    nc.compile()
    return nc


def prep_host_inputs(inputs):
    """Fold BN affine into weights, build diag/scale/bias aux tensors."""
    g = lambda a: np.ascontiguousarray(np.asarray(a, dtype=np.float32))
    wq = (g(inputs["q_g"])[:, None] * g(inputs["q_w"])[:, :, 0, 0]).T
    wkp = (g(inputs["kp_g"])[:, None] * g(inputs["kp_w"])[:, :, 0, 0]).T
    wvp = (g(inputs["vp_g"])[:, None] * g(inputs["vp_w"])[:, :, 0, 0]).T
    wkc = g(inputs["kc_g"])[:, None] * g(inputs["kc_w"])[:, 0].reshape(128, 9)
    wvc = g(inputs["vc_g"])[:, None] * g(inputs["vc_w"])[:, 0].reshape(256, 9)

    diag = np.zeros((3, 9, 128, 128), np.float32)
    for t in range(9):
        diag[0, t] = np.diag(wkc[:, t])
        diag[1, t] = np.diag(wvc[:128, t])
        diag[2, t] = np.diag(wvc[128:, t])

    scale110 = np.zeros(S, np.float32)
    scale110[0] = 1.0 / 9216
    scale110[1:10] = 1.0 / 1024
    scale110[10:46] = 1.0 / 256
    scale110[46:110] = 1.0 / 144
    scl = np.zeros((2, 128, S), np.float32)
    scl[0] = scale110 / 16.0
    scl[1] = scale110

    bias = np.zeros((128, 8), np.float32)
    bias[:, 0] = g(inputs["kp_b"])
    bias[:, 1] = g(inputs["kc_b"])
    bias[:, 2] = g(inputs["vp_b"])[:128]
    bias[:, 3] = g(inputs["vp_b"])[128:]
    bias[:, 4] = g(inputs["vc_b"])[:128]
    bias[:, 5] = g(inputs["vc_b"])[128:]
    bias[:, 6] = g(inputs["q_b"])[:128]
    bias[:, 7] = g(inputs["q_b"])[128:]

    import ml_dtypes
    return {
        "wq": np.ascontiguousarray(wq).astype(ml_dtypes.bfloat16),
        "wkp": np.ascontiguousarray(wkp).astype(ml_dtypes.bfloat16),
        "wvp": np.ascontiguousarray(wvp).astype(ml_dtypes.bfloat16),
        "diag": diag.astype(ml_dtypes.bfloat16),
        "ident": np.eye(128, dtype=ml_dtypes.bfloat16),
        "scl": scl,
        "bias": bias,
    }


def make_in_maps(inputs):
    host = prep_host_inputs(inputs)
    x = np.asarray(inputs["x"], dtype=np.float32)
    B = x.shape[0]
    in_maps = []
    import ml_dtypes
    for b in range(B):
        m = dict(host)
        m["xb"] = np.ascontiguousarray(
            x[b].reshape(512, HW)).astype(ml_dtypes.bfloat16)
        in_maps.append(m)
    return in_maps


_NC = None


def get_nc():
    global _NC
    if _NC is None:
        _NC = build_bass()
    return _NC


def kernel(**inputs):
    from concourse import bass_utils
    nc = get_nc()
    in_maps = make_in_maps(inputs)
    res = bass_utils.run_bass_kernel_spmd(
        nc, in_maps, core_ids=list(range(len(in_maps))), trace=False)
    outs = [np.asarray(r["y"], dtype=np.float32).reshape(512, HH, HH)
            for r in res.results]
    return np.stack(outs, axis=0)



# revision 41
# speedup vs baseline: 2.0330x; 1.2349x over previous
"""CAPAttentionModule Trainium2 kernel.

Data-parallel over batch: 8 images -> 8 NeuronCores, one image per core.
Per core (x: [512, 9216] = [C, H*W], H=W=96):
  k1 = relu(Wkp x + b)              [128, HW]   (1x1 conv, BN folded)
  k2 = relu(dw3x3(k1) + b)          [128, HW]   (depthwise via diagonal matmuls)
  v1 = relu(Wvp x + b)              [256, HW]
  v2 = relu(dw3x3(v1) + b)          [256, HW]
  key = psp([k1;k2])   [256, 110],  value = psp([v1;v2])  [512, 110]
  q  = relu(Wq x + b)               [256, HW]
  sim = softmax_s(q^T key / 16)     [HW, 110]   (no max-subtract; |sim|<4)
  out = x + value @ sim^T           [512, HW]

All matmuls use float32r (full-rate fp32 on the PE at N>=256).
Depthwise 3x3 runs as 9 shifted diagonal matmuls accumulating in PSUM;
SAME-padding comes from a zero column pad (width 98 layout) plus
row-restricted APs at the image top/bottom (has_written overwrite
semantics make ragged accumulation exact).
PSP pooling: one 5D strided reduce to a 24x24 sum grid per map, then
small batched reduces for the 1/3/6/8 grids; normalization (and the
1/sqrt(256) sim scale) is folded into per-s scale tiles.
"""

import numpy as np

P = 128
HH = 96
WP = 98          # padded width/height (zero border ring)
HW = 9216
HWP = WP * WP    # 9604: [98, 98] with zero border, data at [1:97, 1:97]
RB = 24          # row blocks of 4 rows
RBN = 4 * HH     # 384
NCH = 18         # phase-B column chunks
NCW = 512
DWG = 6          # dw row-blocks per psum group
S = 110


def _f32r(ap):
    from concourse import mybir
    return ap.bitcast(mybir.dt.float32r)




def bass_ap_pool_view(ap_rows):
    """[p, >=4*WP] AP at the start of 4 data rows (stride WP) ->
    [p, wq, h, ws] view for a 4x4 pooling reduce over (h, ws)."""
    v = ap_rows[:, 0:4 * WP].rearrange("p (h w) -> p h w", w=WP)
    v = v[:, :, 0:HH]
    return v.rearrange("p h (wq ws) -> p wq h ws", ws=4)

def build_bass():
    import concourse.bacc as bacc
    import concourse.tile as tile
    from concourse import mybir
    from contextlib import ExitStack

    f32 = mybir.dt.float32
    f32r = mybir.dt.float32r
    bf16 = mybir.dt.bfloat16
    AF = mybir.ActivationFunctionType
    AX = mybir.AxisListType

    nc = bacc.Bacc("TRN2", target_bir_lowering=False, debug=False,
                   enable_asserts=False, num_devices=8)

    f8 = mybir.dt.float8e4
    MPM = mybir.MatmulPerfMode
    xb_d = nc.dram_tensor("xb", [512, HW], bf16, kind="ExternalInput").ap()
    xf8_d = nc.dram_tensor("xf8", [512, HW], f8, kind="ExternalInput").ap()
    wkp8_d = nc.dram_tensor("wkp8", [512, 128], f8, kind="ExternalInput").ap()
    wvp8_d = nc.dram_tensor("wvp8", [512, 256], f8, kind="ExternalInput").ap()
    wq_d = nc.dram_tensor("wq", [512, 256], bf16, kind="ExternalInput").ap()
    wkp_d = nc.dram_tensor("wkp", [512, 128], bf16, kind="ExternalInput").ap()
    wvp_d = nc.dram_tensor("wvp", [512, 256], bf16, kind="ExternalInput").ap()
    diag_d = nc.dram_tensor("diag", [3, 9, 128, 128], f8, kind="ExternalInput").ap()
    id_d = nc.dram_tensor("ident", [128, 128], bf16, kind="ExternalInput").ap()
    scl_d = nc.dram_tensor("scl", [2, 128, S], f32, kind="ExternalInput").ap()
    bias_d = nc.dram_tensor("bias", [128, 8], f32, kind="ExternalInput").ap()
    y_d = nc.dram_tensor("y", [512, HW], bf16, kind="ExternalOutput").ap()

    xb_r = xb_d.rearrange("(t p) n -> p t n", p=P)
    xf8_r = xf8_d.rearrange("(t p) n -> p t n", p=P)
    y_r = y_d.rearrange("(t p) n -> p t n", p=P)

    with tile.TileContext(nc) as tc:
        with ExitStack() as top:
            cpool = top.enter_context(tc.tile_pool(name="consts", bufs=1))
            kpool = top.enter_context(tc.tile_pool(name="keep", bufs=1))

            c_wkp = cpool.tile([P, 4 * 128], f8)
            nc.sync.dma_start(c_wkp[:].rearrange("p (t m) -> p t m", t=4),
                              wkp8_d.rearrange("(t p) m -> p t m", p=P))
            c_wvp = cpool.tile([P, 4 * 256], f8)
            nc.sync.dma_start(c_wvp[:].rearrange("p (t m) -> p t m", t=4),
                              wvp8_d.rearrange("(t p) m -> p t m", p=P))
            c_bias = cpool.tile([P, 8], f32)
            nc.sync.dma_start(c_bias[:], bias_d)
            c_wq = cpool.tile([P, 4 * 256], bf16)
            nc.scalar.dma_start(c_wq[:].rearrange("p (t m) -> p t m", t=4),
                                wq_d.rearrange("(t p) m -> p t m", p=P))
            c_dg = cpool.tile([P, 27 * 128], f8)
            nc.scalar.dma_start(c_dg[:].rearrange("p (ct m) -> p ct m", ct=27),
                                diag_d.rearrange("c t p m -> p (c t) m"))
            c_id = cpool.tile([P, 128], bf16)
            nc.scalar.dma_start(c_id[:], id_d)
            c_scl = cpool.tile([P, 2 * S], f32)
            nc.scalar.dma_start(c_scl[:].rearrange("p (s m) -> p s m", s=2),
                                scl_d.rearrange("s p m -> p s m"))

            keyn = kpool.tile([P, 2 * S], bf16)       # normalized key (incl /16)
            vT = kpool.tile([S, 512], bf16)           # value^T [s, c]
            # x (bf16) resident in SBUF for both phases: [p, (t, n)]
            xall = kpool.tile([P, 4 * HW], bf16)
            xall_v = xall[:].rearrange("p (t n) -> p t n", t=4)

            # ---------------- Phase A: key/value branches ----------------
            with ExitStack() as actx:
                bigp = actx.enter_context(tc.tile_pool(name="bigA", bufs=1))
                x8p = actx.enter_context(tc.tile_pool(name="x8", bufs=3))
                blkp = actx.enter_context(tc.tile_pool(name="blk", bufs=6))
                tmpp = actx.enter_context(tc.tile_pool(name="tmpA", bufs=1))

                k1p = bigp.tile([P, HWP], f8)
                v1p = bigp.tile([P, 2 * HWP], f8)
                p24 = bigp.tile([P, 6 * 576], f32)
                allp = bigp.tile([P, 6 * S], f32)
                valn = bigp.tile([P, 4 * S], bf16)

                # zero the pad border (rows 0/97, cols 0/97)
                for chv in (k1p[:, 0:HWP], v1p[:, 0:HWP], v1p[:, HWP:2 * HWP]):
                    c3 = chv.rearrange("p (h w) -> p h w", w=WP)
                    nc.gpsimd.memset(c3[:, 0:1, :], 0.0)
                    nc.gpsimd.memset(c3[:, 97:98, :], 0.0)
                    nc.gpsimd.memset(c3[:, 1:97, 0:1], 0.0)
                    nc.gpsimd.memset(c3[:, 1:97, 97:98], 0.0)

                # primary 1x1 convs, streamed by 4-row blocks (2 blocks/DMA),
                # with per-block pooling of k1/v1a/v1b interleaved on DVE
                with tc.tile_pool(name="psA", bufs=2, space="PSUM") as psA:
                    for rbb in range(RB // 2):
                        nc.sync.dma_start(
                            xall_v[:, :, rbb * 2 * RBN:(rbb + 1) * 2 * RBN],
                            xb_r[:, :, rbb * 2 * RBN:(rbb + 1) * 2 * RBN])
                        xt8 = x8p.tile([P, 4 * 2 * RBN], f8, name="xt8")
                        nc.sync.dma_start(
                            xt8[:].rearrange("p (t n) -> p t n", t=4),
                            xf8_r[:, :, rbb * 2 * RBN:(rbb + 1) * 2 * RBN])
                        xt8_v = xt8[:].rearrange("p (t n) -> p t n", t=4)
                        dsts = [
                            (k1p, 0, c_wkp, 128, 0, 0),
                            (v1p, 0, c_wvp, 256, 2, 2),
                            (v1p, 1, c_wvp, 256, 3, 3),
                        ]
                        for sub in range(2):
                            rb = rbb * 2 + sub
                            for di, (dst, half, wt, wm, bcol, slot) in enumerate(dsts):
                                ps = psA.tile([P, RBN], f32, name=f"pps{di}")
                                wtv = wt[:].rearrange("p (t m) -> p t m", t=4)
                                off = half * 128 if wm == 256 else 0
                                for pr in range(2):
                                    nc.tensor.matmul(
                                        ps[:],
                                        wtv[:, 2 * pr:2 * pr + 2,
                                            off:off + 128],
                                        xt8_v[:, 2 * pr:2 * pr + 2,
                                              sub * RBN:(sub + 1) * RBN],
                                        start=(pr == 0), stop=(pr == 1),
                                        perf_mode=MPM.DoubleRow)
                                dv = dst[:, half * HWP:(half + 1) * HWP].rearrange(
                                    "p (h w) -> p h w", w=WP)
                                nc.scalar.activation(
                                    dv[:, 4 * rb + 1:4 * rb + 5, 1:97],
                                    ps[:].rearrange("p (h w) -> p h w", w=HH),
                                    AF.Relu, bias=c_bias[:, bcol:bcol + 1])
                                st = (4 * rb + 1) * WP + 1
                                pv = dst[:, half * HWP + st:half * HWP + st + 4 * WP]
                                pv = bass_ap_pool_view(pv)
                                nc.vector.reduce_sum(
                                    p24[:, slot * 576 + rb * 24:slot * 576 + (rb + 1) * 24],
                                    pv, axis=AX.XY)

                # small pools over a map range [m0, m1) -> allp columns
                def smallpools(m0, m1):
                    m = m1 - m0
                    allp_v = allp[:, m0 * S:m1 * S].rearrange(
                        "p (m s) -> p m s", s=S)
                    p24s = p24[:, m0 * 576:m1 * 576]
                    nc.vector.reduce_sum(
                        allp_v[:, :, 0:1],
                        p24s.rearrange("p (m s) -> p m s", s=576), axis=AX.X)
                    tmp = tmpp.tile([P, 1152], f32, name="tmp", tag="tmp")
                    nc.vector.reduce_sum(
                        tmp[:, 0:m * 72],
                        p24s.rearrange("p (mh wq ws) -> p mh wq ws", wq=3, ws=8),
                        axis=AX.X)
                    nc.vector.reduce_sum(
                        allp_v[:, :, 1:10],
                        tmp[:, 0:m * 72].rearrange(
                            "p (m hq hs wq) -> p m hq wq hs", m=m, hq=3, hs=8),
                        axis=AX.X)
                    tmp6 = tmpp.tile([P, 1152], f32, name="tmp6", tag="tmp")
                    nc.vector.reduce_sum(
                        tmp6[:, 0:m * 144],
                        p24s.rearrange("p (mh wq ws) -> p mh wq ws", wq=6, ws=4),
                        axis=AX.X)
                    nc.vector.reduce_sum(
                        allp_v[:, :, 10:46],
                        tmp6[:, 0:m * 144].rearrange(
                            "p (m hq hs wq) -> p m hq wq hs", m=m, hq=6, hs=4),
                        axis=AX.X)
                    tmp8 = tmpp.tile([P, 1152], f32, name="tmp8", tag="tmp")
                    nc.vector.reduce_sum(
                        tmp8[:, 0:m * 192],
                        p24s.rearrange("p (mh wq ws) -> p mh wq ws", wq=8, ws=3),
                        axis=AX.X)
                    nc.vector.reduce_sum(
                        allp_v[:, :, 46:110],
                        tmp8[:, 0:m * 192].rearrange(
                            "p (m hq hs wq) -> p m hq wq hs", m=m, hq=8, hs=3),
                        axis=AX.X)


                # depthwise 3x3 via diagonal matmuls + pooling of k2/v2;
                # value maps pooled/transposed as soon as each is complete
                def vt_build(j):
                    tp = psTp.tile([P, 128], bf16, name="tp", tag="tp")
                    nc.tensor.transpose(tp[0:S, :], valn[:, j * S:(j + 1) * S],
                                        c_id[:])
                    nc.scalar.copy(vT[:, j * 128:(j + 1) * 128], tp[0:S, :])

                def val_finish(m0, m1):
                    smallpools(m0, m1)
                    for mm in range(m0, m1):
                        j = mm - 2
                        nc.vector.tensor_mul(valn[:, j * S:(j + 1) * S],
                                             allp[:, mm * S:(mm + 1) * S],
                                             c_scl[:, S:2 * S])
                        vt_build(j)

                with tc.tile_pool(name="psD", bufs=1, space="PSUM") as psD, \
                        tc.tile_pool(name="psTa", bufs=2, space="PSUM") as psTp:
                    # maps 2,3 (v1a, v1b) complete after the primary loop
                    val_finish(2, 4)
                    chunks = [(k1p[:, 0:HWP], 0, 1, 1),
                              (v1p[:, 0:HWP], 1, 4, 4),
                              (v1p[:, HWP:2 * HWP], 2, 5, 5)]

                    def dr_rhs(ch3v, rows0, dx, pstr):
                        # overlapping pair view [p, 2, 4, 96]: two dw taps as
                        # DoubleRow k-tiles (pair stride = row or col shift)
                        v = ch3v[:, rows0:rows0 + 4, dx:dx + HH]
                        vb = v.unsqueeze(1).broadcast_to([P, 2, 4, HH])
                        dims = [list(x) for x in vb.ap]
                        dims[1][0] = pstr
                        vb.ap = mybir.VecI64Pair(dims)
                        return vb

                    # host tap order [0,3, 1,4, 2,5, 6,8, 7]:
                    # ops = (pair slot, row offset, dx, pair stride)
                    drops = [(0, 0, 0, WP), (2, 0, 1, WP),
                             (4, 0, 2, WP), (6, 2, 0, 2)]
                    for chv, ci, bcol, slot in chunks:
                        ch3 = chv.rearrange("p (h w) -> p h w", w=WP)
                        for g in range(RB // DWG):
                            pss = [psD.tile([P, RBN], f32, name=f"dw{j}")
                                   for j in range(DWG)]
                            for oi, (ws, ro, dx, pstr) in enumerate(drops):
                                base = (ci * 9 + ws) * 128
                                wpair = c_dg[:, base:base + 256].rearrange(
                                    "p (pr m) -> p pr m", pr=2)
                                for j in range(DWG):
                                    r0 = (g * DWG + j) * 4
                                    nc.tensor.matmul(
                                        pss[j][:], wpair,
                                        dr_rhs(ch3, r0 + ro, dx, pstr),
                                        start=(oi == 0), stop=False,
                                        perf_mode=MPM.DoubleRow)
                            dg8 = c_dg[:, (ci * 9 + 8) * 128:
                                       (ci * 9 + 9) * 128]
                            for j in range(DWG):
                                r0 = (g * DWG + j) * 4
                                rhs = ch3[:, r0 + 2:r0 + 6, 1:1 + HH]
                                nc.tensor.matmul(pss[j][:], dg8, rhs,
                                                 start=False, stop=True)
                            for j in range(DWG):
                                rb = g * DWG + j
                                blk = blkp.tile([P, RBN], bf16, name="blk")
                                nc.scalar.activation(
                                    blk[:], pss[j][:], AF.Relu,
                                    bias=c_bias[:, bcol:bcol + 1])
                                bv = blk[:].rearrange(
                                    "p (h wq ws) -> p wq h ws", h=4, ws=4)
                                nc.vector.reduce_sum(
                                    p24[:, slot * 576 + rb * 24:slot * 576 + (rb + 1) * 24],
                                    bv, axis=AX.XY)
                        if ci == 0:
                            # key branch done: pool + normalize immediately so
                            # phase-B sim/softmax can overlap the value chunks
                            smallpools(0, 2)
                            for kq in range(2):
                                nc.vector.tensor_mul(
                                    keyn[:, kq * S:(kq + 1) * S],
                                    allp[:, kq * S:(kq + 1) * S], c_scl[:, 0:S])
                        elif ci == 1:
                            val_finish(4, 5)
                        else:
                            val_finish(5, 6)


            # ---------------- Phase B: query / attention / output ----------------
            with ExitStack() as bctx:
                qp = bctx.enter_context(tc.tile_pool(name="qsb", bufs=5))
                pp = bctx.enter_context(tc.tile_pool(name="pexp", bufs=8))
                sp = bctx.enter_context(tc.tile_pool(name="small", bufs=8))
                stp = bctx.enter_context(tc.tile_pool(name="simT", bufs=5))
                obp = bctx.enter_context(tc.tile_pool(name="outb", bufs=3))
                psQ = bctx.enter_context(tc.tile_pool(name="psQ", bufs=1, space="PSUM"))
                psS = bctx.enter_context(tc.tile_pool(name="psS", bufs=2, space="PSUM"))
                psT2 = bctx.enter_context(tc.tile_pool(name="psT2", bufs=2, space="PSUM"))
                psC = bctx.enter_context(tc.tile_pool(name="psC", bufs=2, space="PSUM"))

                for n in range(NCH):
                    qsb = qp.tile([P, 2 * NCW], bf16, name="qsb")
                    for kq in range(2):
                        qps = psQ.tile([P, NCW], f32, name=f"q{kq}")
                        for cc in range(4):
                            lo = cc * 256 + kq * 128
                            nc.tensor.matmul(
                                qps[:], c_wq[:, lo:lo + 128],
                                xall[:, cc * HW + n * NCW:
                                     cc * HW + (n + 1) * NCW],
                                start=(cc == 0), stop=(cc == 3))
                        nc.scalar.activation(qsb[:, kq * NCW:(kq + 1) * NCW],
                                             qps[:], AF.Relu,
                                             bias=c_bias[:, 6 + kq:7 + kq])
                    sT = stp.tile([S, NCW], bf16, name="sT")
                    for ns in range(4):
                        sps = psS.tile([P, S], f32, name="sim")
                        for kq in range(2):
                            nc.tensor.matmul(
                                sps[:],
                                qsb[:, kq * NCW + ns * 128:kq * NCW + (ns + 1) * 128],
                                keyn[:, kq * S:(kq + 1) * S],
                                start=(kq == 0), stop=(kq == 1))
                        pe = pp.tile([P, S], bf16, name="pe")
                        sums = sp.tile([P, 1], f32, name="sums")
                        nc.scalar.activation(pe[:], sps[:], AF.Exp)
                        nc.vector.reduce_sum(sums[:], pe[:], axis=AX.X)
                        rp = sp.tile([P, 1], f32, name="rp")
                        nc.vector.reciprocal(rp[:], sums[:])
                        pn = pp.tile([P, S], bf16, name="pn")
                        nc.vector.tensor_scalar_mul(pn[:], pe[:], rp[:])
                        tp2 = psT2.tile([P, 128], bf16, name="tp2")
                        nc.tensor.transpose(tp2[0:S, :], pn[:], c_id[:])
                        nc.scalar.copy(sT[:, ns * 128:(ns + 1) * 128], tp2[0:S, :])
                    outb = obp.tile([P, 4 * NCW], bf16, name="outb")
                    for cv in range(4):
                        cps = psC.tile([P, NCW], f32, name="ctx")
                        nc.tensor.matmul(cps[:], vT[:, cv * 128:(cv + 1) * 128],
                                         sT[:], start=True, stop=True)
                        nc.vector.tensor_add(outb[:, cv * NCW:(cv + 1) * NCW],
                                             cps[:],
                                             xall[:, cv * HW + n * NCW:
                                                  cv * HW + (n + 1) * NCW])
                    # store on the ScalarE HWDGE ring: keeps a resid-delayed
                    # store from head-of-line blocking the sync-ring x loads
                    nc.scalar.dma_start(
                        y_r[:, :, n * NCW:(n + 1) * NCW],
                        outb[:].rearrange("p (t n) -> p t n", t=4))

    nc.compile()
    return nc


def prep_host_inputs(inputs):
    """Fold BN affine into weights, build diag/scale/bias aux tensors."""
    g = lambda a: np.ascontiguousarray(np.asarray(a, dtype=np.float32))
    wq = (g(inputs["q_g"])[:, None] * g(inputs["q_w"])[:, :, 0, 0]).T
    wkp = (g(inputs["kp_g"])[:, None] * g(inputs["kp_w"])[:, :, 0, 0]).T
    wvp = (g(inputs["vp_g"])[:, None] * g(inputs["vp_w"])[:, :, 0, 0]).T
    wkc = g(inputs["kc_g"])[:, None] * g(inputs["kc_w"])[:, 0].reshape(128, 9)
    wvc = g(inputs["vc_g"])[:, None] * g(inputs["vc_w"])[:, 0].reshape(256, 9)

    diag = np.zeros((3, 9, 128, 128), np.float32)
    order = [0, 3, 1, 4, 2, 5, 6, 8, 7]
    for idx, t in enumerate(order):
        diag[0, idx] = np.diag(wkc[:, t])
        diag[1, idx] = np.diag(wvc[:128, t])
        diag[2, idx] = np.diag(wvc[128:, t])

    scale110 = np.zeros(S, np.float32)
    scale110[0] = 1.0 / 9216
    scale110[1:10] = 1.0 / 1024
    scale110[10:46] = 1.0 / 256
    scale110[46:110] = 1.0 / 144
    scl = np.zeros((2, 128, S), np.float32)
    scl[0] = scale110 / 16.0
    scl[1] = scale110

    bias = np.zeros((128, 8), np.float32)
    bias[:, 0] = g(inputs["kp_b"])
    bias[:, 1] = g(inputs["kc_b"])
    bias[:, 2] = g(inputs["vp_b"])[:128]
    bias[:, 3] = g(inputs["vp_b"])[128:]
    bias[:, 4] = g(inputs["vc_b"])[:128]
    bias[:, 5] = g(inputs["vc_b"])[128:]
    bias[:, 6] = g(inputs["q_b"])[:128]
    bias[:, 7] = g(inputs["q_b"])[128:]

    import ml_dtypes
    return {
        "wq": np.ascontiguousarray(wq).astype(ml_dtypes.bfloat16),
        "wkp": np.ascontiguousarray(wkp).astype(ml_dtypes.bfloat16),
        "wvp": np.ascontiguousarray(wvp).astype(ml_dtypes.bfloat16),
        "diag": diag.astype(ml_dtypes.float8_e4m3),
        "ident": np.eye(128, dtype=ml_dtypes.bfloat16),
        "scl": scl,
        "bias": bias,
    }


def make_in_maps(inputs):
    host = prep_host_inputs(inputs)
    x = np.asarray(inputs["x"], dtype=np.float32)
    B = x.shape[0]
    in_maps = []
    import ml_dtypes
    for b in range(B):
        m = dict(host)
        m["xb"] = np.ascontiguousarray(
            x[b].reshape(512, HW)).astype(ml_dtypes.bfloat16)
        in_maps.append(m)
    return in_maps


_NC = None


def get_nc():
    global _NC
    if _NC is None:
        _NC = build_bass()
    return _NC


def kernel(**inputs):
    from concourse import bass_utils
    nc = get_nc()
    in_maps = make_in_maps(inputs)
    res = bass_utils.run_bass_kernel_spmd(
        nc, in_maps, core_ids=list(range(len(in_maps))), trace=False)
    outs = [np.asarray(r["y"], dtype=np.float32).reshape(512, HH, HH)
            for r in res.results]
    return np.stack(outs, axis=0)

